# revision 56
# baseline (speedup 1.0000x reference)
"""Multi-head scaled-cosine attention (B=2, L=2048, E=2048, H=16, D=128) on 8 trn2 cores.

Sharding: core c = (b, g) with b = batch (2), g = head-group of 4 heads (4 groups).
Each core computes its 4 heads' attention for its batch plus the partial output
projection; the host sums the 4 per-group partials per batch.

Precision: matmuls run in bf16 (PSUM accumulation is fp32). The Q/K RMS-norm
cancels exactly under the subsequent L2 normalization; the L2 reciprocal (and
logit scale) are folded into a diagonal matrix applied by the PE transpose
that produces Q^T/K^T. exp(bias - rowmax) is precomputed on the host and
folded in multiplicatively. Scores are built directly in [k, q] orientation so
softmax and attn@V need no on-chip transpose of the probability matrix. Q/K
head dims are host-permuted (evens|odds) so RoPE uses contiguous vector ops;
the permutation cancels inside q.k.

Softmax denominator: adjacent k-tiles' probability tiles are pair-summed on
DVE right behind the exp*bias mults, halving the columns the ones-matmul must
stream on the PE; the ones-matmuls run as a late burst (kt14/15 + loop end,
the last two k-tiles raw) so no PSUM bank is resident during the loop and the
Ln/exp(-ln) 1/den chain (and its activation-table loads) completes inside the
next chunk's exp-free warm window.

Scheduling: the first kv x-block is split across four engine DMA queues and
posted ahead of the weights, and the first V-projection runs as two l-half
chains, so the PE starts within a few us. Q^T/K^T transposes run 1-2 l-tiles
behind their norm chain (square+reduce on DVE so only the sqrt is scalar).
The last eight Q l-tiles' normalize tails are deferred into the first
attention q-chunk (their 8 sqrts batched into one scalar op, the vector tails
paced every other k-tile, transposes one k-tile behind). Every q-chunk's
first two k-tiles (scores+exp+bias-mult) are computed in the shadow of the
previous phase ("warm"), so the in-order PE queue crosses phase and chunk
boundaries without waiting on the scalar/DVE pipelines; PV runs 3+ k-tiles
behind scores, and outproj tiles of the previous chunk drain one per k-tile
as PE filler.
"""
import sys
sys.path.insert(0, '/opt/trn_rl_repo')
import math
import numpy as np
import ml_dtypes

import concourse.bacc as bacc
import concourse.mybir as mybir
import concourse.tile as tile
from concourse.bass_utils import run_bass_kernel_spmd

F32 = mybir.dt.float32
F32R = mybir.dt.float32r
BF16 = mybir.dt.bfloat16
NP_BF16 = ml_dtypes.bfloat16
ALU = mybir.AluOpType
AF = mybir.ActivationFunctionType

B, L, E, H, D = 2, 2048, 2048, 16, 128
G = 4                 # head groups
HPG = H // G          # heads per group = 4
GD = HPG * D          # 512, per-group projection width
P = 128               # partitions
NLT = L // P          # 16 l-tiles
NET = E // P          # 16 e-tiles (contraction)
NQC = L // 512        # 4 q-chunks
NKT = L // P          # 16 k-tiles
HD2 = GD // 2         # 256
LOGIT_SCALE_MAX = math.log(1.0 / 0.01)


def _build(apply_qs: bool, apply_ks: bool):
    nc = bacc.Bacc(None, target_bir_lowering=False)
    d = {}
    # x inputs in [p, l, e16] layout: per partition, per l, 16 contiguous
    # e-chunk entries -> one 4KB contiguous run per partition per l-block DMA.
    d['xqT'] = nc.dram_tensor("xqT", [P, L * NET], BF16, kind="ExternalInput")
    d['xkvT'] = nc.dram_tensor("xkvT", [P, L * NET], BF16, kind="ExternalInput")
    d['expBT'] = nc.dram_tensor("expBT", [L, L], BF16, kind="ExternalInput")
    d['wqT'] = nc.dram_tensor("wqT", [E, GD], BF16, kind="ExternalInput")
    d['wkT'] = nc.dram_tensor("wkT", [E, GD], BF16, kind="ExternalInput")
    d['wvT'] = nc.dram_tensor("wvT", [E, GD], BF16, kind="ExternalInput")
    d['woS'] = nc.dram_tensor("woS", [GD, E], BF16, kind="ExternalInput")
    d['csq'] = nc.dram_tensor("csq", [L, GD], F32, kind="ExternalInput")
    d['csk'] = nc.dram_tensor("csk", [L, GD], F32, kind="ExternalInput")
    d['ls'] = nc.dram_tensor("ls", [P, HPG], F32, kind="ExternalInput")
    if apply_qs:
        d['qscale'] = nc.dram_tensor("qscale", [P, GD], F32, kind="ExternalInput")
    if apply_ks:
        d['kscale'] = nc.dram_tensor("kscale", [P, GD], F32, kind="ExternalInput")
    out = nc.dram_tensor("out", [L, E], BF16, kind="ExternalOutput")

    with tile.TileContext(nc) as tc:
        with tc.tile_pool(name="persist", bufs=1) as persist:
            qT = persist.tile([P, HPG, L], BF16, tag="qT", name="qT")
            kT = persist.tile([P, HPG, L], BF16, tag="kT", name="kT")
            v_sb = persist.tile([P, NLT, GD], BF16, tag="v_sb")
            identb = persist.tile([P, P], BF16, tag="identb")
            identf = persist.tile([P, P], F32, tag="identf")
            nc.vector.memset(identf[:], 0.0)
            nc.gpsimd.affine_select(out=identf[:], in_=identf[:],
                                    compare_op=ALU.not_equal, fill=1.0, base=0,
                                    pattern=[[-1, P]], channel_multiplier=1)
            nc.vector.tensor_copy(identb[:], identf[:])
            ones_f = persist.tile([P, P], F32, tag="ones_f")
            nc.vector.memset(ones_f[:], 1.0)
            ones_r = persist.tile([P, P], F32R, tag="ones_r")
            nc.scalar.copy(ones_r[:], ones_f[:])
            ones_b = persist.tile([P, P], BF16, tag="ones_b")
            nc.vector.tensor_copy(ones_b[:], ones_f[:])
            ls_t = persist.tile([P, HPG], F32, tag="ls_t")

            wo_sb = persist.tile([P, HPG, E], BF16, tag="wo_sb")

            qs_t = ks_t = None
            if apply_qs:
                qs_t = persist.tile([P, GD], F32, tag="qs_t")
            if apply_ks:
                ks_t = persist.tile([P, GD], F32, tag="ks_t")

            q3_hold = persist.tile([P, 8, GD], BF16, tag="q3_hold")
            nrm_hold = persist.tile([P, 8 * HPG], F32, tag="nrm_hold")
            sq_hold = persist.tile([P, 8 * HPG], F32, tag="sq_hold")
            # first attention q-chunk's kt=0,1 probability tiles, computed in
            # the shadow of the Q-projection tail so the PE never idles at the
            # projection->attention boundary
            warm_pt = persist.tile([P, 2, 2048], BF16, tag="warm_pt")
            warm_eb = persist.tile([P, 2, 512], BF16, tag="warm_eb")

            from contextlib import ExitStack
            proj_ctx = ExitStack()
            sbp = proj_ctx.enter_context(tc.tile_pool(name="proj_sb", bufs=7))
            wpool = proj_ctx.enter_context(tc.tile_pool(name="proj_w", bufs=1))
            w_all = {}
            for wname in ('wvT', 'wkT', 'wqT'):
                w_all[wname] = wpool.tile([P, NET, GD], BF16, tag=wname,
                                          name=f"w_{wname}")
            nrm = proj_ctx.enter_context(tc.tile_pool(name="proj_nrm", bufs=6))
            psp = proj_ctx.enter_context(tc.tile_pool(name="proj_ps", bufs=4, space="PSUM"))
            pst = proj_ctx.enter_context(tc.tile_pool(name="proj_pst", bufs=2, space="PSUM"))

            # ---- head DMAs: first x blocks (queue-split) ahead of weights ----
            blks = {}

            def load_blk(key, lt, x_dram, split=1, engines=None):
                blk = sbp.tile([P, P, NET], BF16, tag="xblk", name=f"xblk_{key}_{lt}")
                ch = P // split
                engines = engines or [nc.sync] * split
                for s in range(split):
                    engines[s].dma_start(
                        blk[:, s * ch:(s + 1) * ch, :],
                        x_dram[:, (lt * P + s * ch) * NET:(lt * P + (s + 1) * ch) * NET]
                            .rearrange("p (l e) -> p l e", e=NET))
                blks[(key, lt)] = blk

            def load_w(wname, e0, e1):
                nc.sync.dma_start(
                    w_all[wname][:, e0:e1, :],
                    d[wname][e0 * P:e1 * P, :].rearrange("(e p) c -> p e c", p=P))

            nc.scalar.dma_start(w_all['wvT'][:, 0:1, :],
                                d['wvT'][0:P, :].rearrange("(e p) c -> p e c", p=P))
            load_blk('kv', 0, d['xkvT'], split=8,
                     engines=[nc.sync, nc.gpsimd, nc.scalar, nc.sync,
                              nc.gpsimd, nc.scalar, nc.sync, nc.gpsimd])
            load_blk('kv', 1, d['xkvT'], split=2,
                     engines=[nc.gpsimd, nc.sync])
            load_w('wvT', 1, 4)
            load_w('wvT', 4, 10)
            load_w('wvT', 10, 16)
            kv_loaded = 2
            for g4 in range(4):
                load_w('wkT', 4 * g4, 4 * g4 + 4)
            nc.sync.dma_start(ls_t[:], d['ls'][:])

            def qk_norm(lt, psum, cs_dram, scale_tile, use_ls, q3_out=None,
                        nrm_out=None, ve=None, ce=None):
                """RoPE + L2-normalize one l-tile; returns q3 [P, GD] bf16.
                With nrm_out set, stops after the sqrt (writing |q| there and
                the un-normalized rotated values to q3_out) so the normalize
                tail can be emitted later without blocking the scalar queue.
                ve picks the engine for the heavy elementwise ops (DVE by
                default; GpSimd for late held tiles to clear DVE backlog)."""
                ve = ve or nc.vector
                q1 = nrm.tile([P, GD], BF16, tag="q1")
                if ce is None:
                    nc.scalar.copy(q1[:], psum)
                else:
                    ce.tensor_copy(q1[:], psum)
                if scale_tile is not None:
                    nc.vector.tensor_mul(q1[:], q1[:], scale_tile[:])
                cst = nrm.tile([P, GD], F32, tag="cst")
                nc.sync.dma_start(cst[:], cs_dram[lt * P:(lt + 1) * P, :])
                ct, st = cst[:, :HD2], cst[:, HD2:]
                # per-head layout [evens(64) | odds(64)] (host-permuted weights)
                q1v = q1[:].rearrange("p (hh par dd) -> p hh par dd", hh=HPG, par=2)
                qe, qo = q1v[:, :, 0, :], q1v[:, :, 1, :]
                q2 = q3_out if q3_out is not None \
                    else nrm.tile([P, GD], BF16, tag="q2", name="q2")[:]
                q2v = q2.rearrange("p (hh par dd) -> p hh par dd", hh=HPG, par=2)
                re, ro = q2v[:, :, 0, :], q2v[:, :, 1, :]
                ctv = ct.rearrange("p (hh dd) -> p hh dd", hh=HPG)
                stv = st.rearrange("p (hh dd) -> p hh dd", hh=HPG)
                tmp = nrm.tile([P, HD2], BF16, tag="tmp")
                tv = tmp[:].rearrange("p (hh dd) -> p hh dd", hh=HPG)
                # evens: qe*c - qo*s ; odds: qo*c + qe*s
                ve.tensor_tensor(tv, qo, stv, ALU.mult)
                ve.tensor_tensor(re, qe, ctv, ALU.mult)
                ve.tensor_sub(re, re, tv)
                ve.tensor_tensor(tv, qe, stv, ALU.mult)
                ve.tensor_tensor(ro, qo, ctv, ALU.mult)
                ve.tensor_add(ro, ro, tv)
                # L2 norm over each head's (now contiguous) D slice; the
                # square+reduce stays off the scalar queue so only the sqrt
                # is scalar
                sqs = nrm.tile([P, GD], BF16, tag="sqs")
                ve.tensor_tensor(sqs[:], q2, q2, ALU.mult)
                if nrm_out is not None:
                    # held tile: stash the raw sum of squares; all 8 sqrts run
                    # as one batched scalar op inside the first q-chunk
                    nc.vector.tensor_reduce(
                        nrm_out, sqs[:].rearrange("p (hh dd) -> p hh dd", hh=HPG),
                        mybir.AxisListType.X, ALU.add)
                    return None
                acc = nrm.tile([P, HPG], F32, tag="acc")
                nc.vector.tensor_reduce(
                    acc[:], sqs[:].rearrange("p (hh dd) -> p hh dd", hh=HPG),
                    mybir.AxisListType.X, ALU.add)
                nrm_t = nrm.tile([P, HPG], F32, tag="nrm_t", name="nrm_t")[:]
                nc.scalar.activation(nrm_t, acc[:], AF.Sqrt)
                nc.vector.tensor_scalar_max(nrm_t, nrm_t, 1e-12)
                rcp = nrm.tile([P, HPG], F32, tag="rcp")
                nc.vector.reciprocal(rcp[:], nrm_t)
                if use_ls:
                    nc.vector.tensor_mul(rcp[:], rcp[:], ls_t[:])
                q3 = nrm.tile([P, GD], BF16, tag="q3", name="q3")[:]
                for h in range(HPG):
                    nc.vector.tensor_scalar_mul(q3[:, h * D:(h + 1) * D],
                                                q2[:, h * D:(h + 1) * D], rcp[:, h:h + 1])
                return q3

            pend_tr = []

            def emit_transposes(pend, pool):
                lt, q3, dstT = pend
                pt = pool.tile([P, GD], BF16, tag="pt" if pool is pst else "sh",
                               name=f"pt_{lt}")
                for h in range(HPG):
                    nc.tensor.matmul(pt[:, h * D:(h + 1) * D], q3[:, h * D:(h + 1) * D],
                                     identb[:], is_transpose=True)
                nc.any.tensor_copy(
                    dstT[:, :, lt * P:(lt + 1) * P],
                    pt[:].rearrange("p (h dd) -> p h dd", h=HPG))

            # merged V+K phase: one xkvT block load feeds both projections
            for lt in range(NLT):
                while kv_loaded <= lt + 5 and kv_loaded < NLT:
                    load_blk('kv', kv_loaded, d['xkvT'])
                    kv_loaded += 1
                if lt == 6:
                    for g4 in range(4):
                        load_w('wqT', 4 * g4, 4 * g4 + 4)
                    if apply_qs:
                        nc.sync.dma_start(qs_t[:], d['qscale'][:])
                    if apply_ks:
                        nc.sync.dma_start(ks_t[:], d['kscale'][:])
                if lt == 10:
                    nc.sync.dma_start(
                        wo_sb[:], d['woS'][:].rearrange("(h p) e -> p h e", p=P))
                blk = blks.pop(('kv', lt))
                psum_v = psp.tile([P, GD], F32, tag="psum", name=f"psumv_{lt}")
                if lt == 0:
                    for e in range(NET):
                        nc.tensor.matmul(psum_v[0:64, :], blk[:, 0:64, e],
                                         w_all['wvT'][:, e, :],
                                         start=(e == 0), stop=(e == NET - 1))
                    for e in range(NET):
                        nc.tensor.matmul(psum_v[64:128, :], blk[:, 64:128, e],
                                         w_all['wvT'][:, e, :],
                                         start=(e == 0), stop=(e == NET - 1))
                else:
                    for e in range(NET):
                        nc.tensor.matmul(psum_v[:], blk[:, :, e],
                                         w_all['wvT'][:, e, :],
                                         start=(e == 0), stop=(e == NET - 1))
                psum_k = psp.tile([P, GD], F32, tag="psum", name=f"psumk_{lt}")
                for e in range(NET):
                    nc.tensor.matmul(psum_k[:], blk[:, :, e], w_all['wkT'][:, e, :],
                                     start=(e == 0), stop=(e == NET - 1))
                while len(pend_tr) > 1:
                    emit_transposes(pend_tr.pop(0), pst)
                nc.scalar.copy(v_sb[:, lt, :], psum_v[:])
                q3 = qk_norm(lt, psum_k[:], d['csk'], ks_t, False)
                pend_tr.append((lt, q3, kT))

            def warm_phase(nqc, kt, ph, pool):
                """Emit one slice of the (nqc, kt<2) attention warmup: scores,
                exp and bias-mult run in the shadow of the previous phase so
                the epilogue's scalar table loads never gate them."""
                ptw = warm_pt[:, kt, :]
                if ph == 0:
                    nc.sync.dma_start(
                        warm_eb[:, kt, :],
                        d['expBT'][kt * P:(kt + 1) * P, nqc * 512:(nqc + 1) * 512])
                    hs = (0, 1)
                elif ph == 1:
                    hs = (2, 3)
                else:
                    hs = ()
                for h in hs:
                    s_ps = pool.tile([P, GD], F32,
                                     tag="s_w" if pool is pst else "sh",
                                     name=f"sw{nqc}_{kt}_{h}")
                    nc.tensor.matmul(s_ps[:], kT[:, h, kt * P:(kt + 1) * P],
                                     qT[:, h, nqc * 512:(nqc + 1) * 512],
                                     start=True, stop=True)
                    nc.scalar.activation(ptw[:, h * 512:(h + 1) * 512], s_ps[:],
                                         AF.Exp)
                if ph in (1, 2):
                    hpp = ph - 1
                    ebb = warm_eb[:, kt, :].rearrange(
                        "p (o q) -> p o q", o=1).broadcast_to([P, 2, 512])
                    half = ptw[:, hpp * 1024:(hpp + 1) * 1024].rearrange(
                        "p (i q) -> p i q", i=2)
                    nc.vector.tensor_tensor(half, half, ebb, ALU.mult)

            # process the l-tiles needed last (by the last q-chunk) FIRST so
            # the end-of-loop norm-chain drain only delays tiles nobody needs
            # soon; hold the last 8 tiles' normalize tails back as PE filler
            # for the first attention q-chunk
            q_order = list(range(12, 16)) + list(range(12))
            q_loaded = 2
            held_slots = []
            for i in range(2):
                load_blk('q', q_order[i], d['xqT'])
            for i, lt in enumerate(q_order):
                while q_loaded <= i + 5 and q_loaded < NLT:
                    load_blk('q', q_order[q_loaded], d['xqT'])
                    q_loaded += 1
                blk = blks.pop(('q', lt))
                psum_q = psp.tile([P, GD], F32, tag="psum", name=f"psumq_{lt}")
                for e in range(NET):
                    nc.tensor.matmul(psum_q[:], blk[:, :, e], w_all['wqT'][:, e, :],
                                     start=(e == 0), stop=(e == NET - 1))
                while len(pend_tr) > 2:
                    emit_transposes(pend_tr.pop(0), pst)
                if i in (9, 10) and pend_tr:
                    # drain the remaining deferred transposes early so their
                    # PSUM-freeing copies don't gate the first attention scores
                    emit_transposes(pend_tr.pop(0), pst)
                if i >= 11:
                    for kt_w, ph_w in ((0, i - 11), (1, i - 13)):
                        if 0 <= ph_w <= 2:
                            warm_phase(0, kt_w, ph_w, pst)
                if i >= 8:
                    j = i - 8
                    qk_norm(lt, psum_q[:], d['csq'], qs_t, True,
                            q3_out=q3_hold[:, j, :],
                            nrm_out=nrm_hold[:, j * HPG:(j + 1) * HPG])
                    held_slots.append((lt, j))
                else:
                    q3 = qk_norm(lt, psum_q[:], d['csq'], qs_t, True)
                    pend_tr.append((lt, q3, qT))
            while pend_tr:
                emit_transposes(pend_tr.pop(0), pst)
            proj_ctx.close()

            # attention per q-chunk
            att_ctx = ExitStack()
            asb = att_ctx.enter_context(tc.tile_pool(name="att_sb", bufs=3))
            atp = att_ctx.enter_context(tc.tile_pool(name="att_at", bufs=1))
            aop = att_ctx.enter_context(tc.tile_pool(name="att_o", bufs=4))
            ps_pv = att_ctx.enter_context(tc.tile_pool(name="ps_pv", bufs=1, space="PSUM"))
            ps_sh = att_ctx.enter_context(tc.tile_pool(name="ps_sh", bufs=4, space="PSUM"))

            out_tasks = []
            pend_norm = None
            held_ready = []

            def emit_out_task(task, final=False):
                oqc, attn, lsub, ec = task
                o_ps = ps_sh.tile([P, 512], F32, tag="sh", name=f"o{oqc}_{lsub}_{ec}")
                for h in range(HPG):
                    nc.tensor.matmul(o_ps[:], attn[h][:, lsub * P:(lsub + 1) * P],
                                     wo_sb[:, h, ec * 512:(ec + 1) * 512],
                                     start=(h == 0), stop=(h == HPG - 1))
                o_sb = aop.tile([P, 512], BF16, tag="o_sb", name=f"ob{oqc}_{lsub}_{ec}")
                if final and (lsub + ec) % 2 == 1:
                    nc.scalar.copy(o_sb[:], o_ps[:])
                else:
                    nc.vector.tensor_copy(o_sb[:], o_ps[:])
                nc.sync.dma_start(
                    out[oqc * 512 + lsub * P: oqc * 512 + (lsub + 1) * P,
                        ec * 512:(ec + 1) * 512], o_sb[:])

            def emit_held_dve(slot):
                """Deferred (vector-only) normalize tail for a held Q l-tile."""
                hlt, j = slot
                nv = sq_hold[:, j * HPG:(j + 1) * HPG]
                rcp = asb.tile([P, HPG], F32, tag="hrcp", name=f"hrcp{j}", bufs=2)
                nc.vector.tensor_scalar_max(rcp[:], nv, 1e-12)
                nc.vector.reciprocal(rcp[:], rcp[:])
                nc.vector.tensor_mul(rcp[:], rcp[:], ls_t[:])
                q3 = q3_hold[:, j, :]
                for h in range(HPG):
                    nc.vector.tensor_scalar_mul(q3[:, h * D:(h + 1) * D],
                                                q3[:, h * D:(h + 1) * D],
                                                rcp[:, h:h + 1])
                return (hlt, q3)

            def emit_den_group(den_t, dg, start, stop):
                for h in range(HPG):
                    nc.tensor.matmul(den_t[32 * h:32 * h + 32, :],
                                     ones_b[:, 0:32],
                                     dg[:, h * 512:(h + 1) * 512],
                                     start=start, stop=stop,
                                     tile_position=(0, 32 * h))

            def emit_epilogue(pend):
                """Ln + exp(-x) chain for a q-chunk's 1/den; the scalar table
                loads land in the exp-free warm window of the next chunk."""
                pqc = pend['qc']
                lnd = asb.tile([P, 512], F32, tag="lnd", name=f"lnd{pqc}", bufs=2)
                nc.scalar.activation(lnd[:], pend['den'][:], AF.Ln)
                rcp32s = []
                for i in range(2):
                    r32 = asb.tile([64, 512], F32R, tag=f"rcp32{i}",
                                   name=f"rcp32{pqc}_{i}", bufs=2)
                    nc.scalar.activation(r32[:], lnd[64 * i:64 * i + 64, :],
                                         AF.Exp, scale=-1.0)
                    rcp32s.append(r32)
                pend['rcp32s'] = rcp32s

            def emit_norm(pend):
                """Broadcast 1/den across partitions and normalize pv -> attn;
                returns the outproj tile tasks for this q-chunk."""
                nqc, npvc, rcp32s = pend['qc'], pend['pvc'], pend['rcp32s']
                attn = [atp.tile([P, 512], BF16, tag=f"at{h}", name=f"at{nqc}_{h}",
                                 bufs=2) for h in range(HPG)]
                for h in range(HPG):
                    b_ps = ps_sh.tile([P, 512], F32, tag="sh", name=f"b{nqc}_{h}")
                    off = 32 * (h % 2)
                    nc.tensor.matmul(b_ps[:], ones_r[off:off + 1, :],
                                     rcp32s[h // 2][off:off + 1, :],
                                     start=True, stop=True)
                    rcpb = asb.tile([P, 512], BF16, tag="rcpb", name=f"rb{nqc}_{h}")
                    nc.vector.tensor_copy(rcpb[:], b_ps[:])
                    nc.vector.tensor_mul(attn[h][:], npvc[h], rcpb[:])
                return [(nqc, attn, lsub, ec)
                        for lsub in range(4) for ec in range(4)]

            for qc in range(NQC):
                qsl = slice(qc * 512, (qc + 1) * 512)
                last = qc == NQC - 1
                pv = [ps_pv.tile([P, 512], F32, tag=f"pv{h}", name=f"pv{qc}_{h}")
                      for h in range(HPG)]

                def stage1(kt):
                    eb = asb.tile([P, 512], BF16, tag="eb", name=f"eb{qc}_{kt}")
                    nc.sync.dma_start(eb[:], d['expBT'][kt * P:(kt + 1) * P, qsl])
                    p_t = asb.tile([P, 2048], BF16, tag="p_t", name=f"pt{qc}_{kt}", bufs=5)
                    ebb = eb[:].rearrange("p (o q) -> p o q", o=1).broadcast_to([P, 2, 512])
                    return p_t, ebb

                def score_head(kt, p_t, h):
                    s_ps = ps_sh.tile([P, 512], F32, tag="sh", name=f"sp{qc}_{kt}_{h}")
                    nc.tensor.matmul(s_ps[:], kT[:, h, kt * P:(kt + 1) * P],
                                     qT[:, h, qsl], start=True, stop=True)
                    nc.scalar.activation(p_t[:, h * 512:(h + 1) * 512], s_ps[:], AF.Exp)

                def mult_half(p_t, ebb, hpp):
                    half = p_t[:, hpp * 1024:(hpp + 1) * 1024].rearrange(
                        "p (i q) -> p i q", i=2)
                    nc.vector.tensor_tensor(half, half, ebb, ALU.mult)

                def stage2_pv(kt, p_t):
                    for h in range(HPG):
                        nc.tensor.matmul(pv[h][:], v_sb[:, kt, h * D:(h + 1) * D],
                                         p_t[:, h * 512:(h + 1) * 512],
                                         start=(kt == 0), stop=(kt == NKT - 1))

                # den pre-reduction: pairs of k-tiles are summed on DVE right
                # behind the exp*bias mults; the ones-matmuls run as a burst
                # in the next chunk's warm window (no resident PSUM bank, and
                # the Ln/Exp table thrash lands where the scalar queue is idle)
                dacc = {}
                quad = {}
                grp_first = None
                den_t = None

                s2q = []
                for kt in range(NKT):
                    warm = kt < 2
                    if warm:
                        # scores/exp/mult for these tiles ran in the shadow of
                        # the previous phase
                        p_t = warm_pt[:, kt, :]
                    else:
                        p_t, ebb = stage1(kt)
                        score_head(kt, p_t, 0)
                        score_head(kt, p_t, 1)
                        mult_half(p_t, ebb, 0)
                        score_head(kt, p_t, 2)
                    # PE fillers go BEFORE the stage2 pop so the in-order PE
                    # queue has ready work while exp/mult of prev completes
                    if pend_norm is not None and kt == 5:
                        out_tasks.extend(emit_norm(pend_norm))
                        pend_norm = None
                    if kt == 15:
                        for h in range(HPG):
                            nc.tensor.matmul(
                                den_t[32 * h:32 * h + 32, :],
                                ones_b[:, 0:32],
                                s2q[-1][1][:, h * 512:(h + 1) * 512],
                                start=False, stop=False,
                                tile_position=(0, 32 * h))
                    if held_ready and kt >= 5 and kt % 2 == 1:
                        emit_transposes(held_ready.pop(0) + (qT,), ps_sh)
                    for _ in range(2 if kt < 2 else 1):
                        if out_tasks:
                            emit_out_task(out_tasks.pop(0))
                    # PV runs 2+ k-tiles behind scores so the cross-engine
                    # score->exp->mult chain latency never stalls the PE; qc0
                    # lags more so the projection-phase queue drain hides
                    lag = 4 if (qc == 0 and kt < 8) else 3
                    if len(s2q) >= lag:
                        stage2_pv(*s2q.pop(0))
                    if not warm:
                        score_head(kt, p_t, 3)
                        mult_half(p_t, ebb, 1)
                    if held_slots and kt == 2:
                        nc.scalar.activation(sq_hold[:], nrm_hold[:], AF.Sqrt)
                    if held_slots and kt >= 4 and kt % 2 == 0:
                        held_ready.append(emit_held_dve(held_slots.pop(0)))
                    if qc < NQC - 1 and kt >= 12:
                        for kt_w, ph_w in ((0, kt - 12), (1, kt - 13)):
                            if 0 <= ph_w <= 2:
                                warm_phase(qc + 1, kt_w, ph_w, ps_sh)
                    g, r = divmod(kt, 2)
                    if kt >= 14:
                        pass  # raw den matmuls, no pair-add needed
                    elif r == 0:
                        grp_first = p_t
                    else:
                        dg = asb.tile([P, 2048], BF16, tag="dacc",
                                      name=f"dacc{qc}_{g}", bufs=8)
                        dacc[g] = dg
                        nc.vector.tensor_add(dg[:], grp_first[:], p_t[:])
                        if g % 2 == 1 and g < 6:
                            # second-level quad sums on the idle GpSimd: the
                            # kt14 burst is 3+ k-tiles away, so its latency
                            # never reaches the PE queue
                            qt_ = asb.tile([P, 2048], BF16, tag="qacc",
                                           name=f"qacc{qc}_{g // 2}", bufs=3)
                            quad[g // 2] = qt_
                            nc.gpsimd.tensor_add(qt_[:], dacc[g - 1][:], dg[:])
                    if kt == 14:
                        # den burst late in the loop: no resident PSUM bank
                        # before this, and the allocation sits deep enough in
                        # the rotation that next-chunk scores never WAR on
                        # the Ln read
                        den_t = ps_sh.tile([P, 512], F32, tag="sh",
                                           name=f"den{qc}")
                        for i_g, dg_ in enumerate([quad[0], quad[1], quad[2],
                                                   dacc[6]]):
                            emit_den_group(den_t, dg_, i_g == 0, False)
                    s2q.append((kt, p_t))
                for item in s2q:
                    stage2_pv(*item)
                for h in range(HPG):
                    nc.tensor.matmul(den_t[32 * h:32 * h + 32, :],
                                     ones_b[:, 0:32],
                                     s2q[-1][1][:, h * 512:(h + 1) * 512],
                                     start=False, stop=True,
                                     tile_position=(0, 32 * h))
                pend = {'qc': qc, 'den': den_t}
                emit_epilogue(pend)
                if not last:
                    pvc = []
                    for h in range(HPG):
                        c = asb.tile([P, 512], BF16, tag=f"pvc{h}",
                                     name=f"pvc{qc}_{h}", bufs=2)
                        nc.vector.tensor_copy(c[:], pv[h][:])
                        pvc.append(c[:])
                    pend['pvc'] = pvc
                    pend_norm = pend
                else:
                    pend['pvc'] = [pv[h][:] for h in range(HPG)]
                    # drain carried tasks first: they fill the PE while the
                    # Ln/Exp chain and table loads run
                    while out_tasks:
                        emit_out_task(out_tasks.pop(0))
                    out_tasks.extend(emit_norm(pend))
            while out_tasks:
                emit_out_task(out_tasks.pop(0), final=True)
            att_ctx.close()
    nc.compile()
    return nc


# head-dim permutation: within each head, evens first then odds
_PERM = np.empty(GD, np.int64)
for _i in range(GD):
    _h, _j = divmod(_i, D)
    _par, _dd = divmod(_j, D // 2)
    _PERM[_i] = _h * D + 2 * _dd + _par


def _x_relayout(x):
    # [L, E] f32 -> [P, L*NET] bf16 with column index l*NET + e  (4KB runs)
    xt = np.ascontiguousarray(x.T.astype(NP_BF16))        # [E, L]
    xr = xt.reshape(NET, P, L).transpose(1, 2, 0)          # [P, L, NET]
    return np.ascontiguousarray(xr.reshape(P, L * NET))


def _prepare(inputs):
    f32 = np.float32
    inputs_q = np.asarray(inputs["inputs_q"], f32)
    inputs_kv = np.asarray(inputs["inputs_kv"], f32)
    bias = np.asarray(inputs["bias"], f32).reshape(L, L)
    q_sin = np.asarray(inputs["q_sinusoids"], f32)
    k_sin = np.asarray(inputs["k_sinusoids"], f32)
    Wq = np.asarray(inputs["Wq"], f32)
    Wk = np.asarray(inputs["Wk"], f32)
    Wv = np.asarray(inputs["Wv"], f32)
    Wo = np.asarray(inputs["Wo"], f32)
    qns = np.asarray(inputs["q_norm_scale"], f32)
    kns = np.asarray(inputs["k_norm_scale"], f32)
    ls = np.asarray(inputs["logit_scale"], f32)

    apply_qs = not np.all(qns == 1.0)
    apply_ks = not np.all(kns == 1.0)

    bm = bias.max(axis=1, keepdims=True)
    expBT = np.ascontiguousarray(np.exp((bias - bm).T).astype(NP_BF16))
    ls_e = np.exp(np.minimum(ls, LOGIT_SCALE_MAX)).astype(f32)

    per_b = []
    for b in range(B):
        per_b.append(dict(
            xqT=_x_relayout(inputs_q[b]),
            xkvT=_x_relayout(inputs_kv[b]),
            csq=np.ascontiguousarray(np.concatenate(
                [np.tile(q_sin[b][:, 0::2], (1, HPG)),
                 np.tile(q_sin[b][:, 1::2], (1, HPG))], axis=1)),
            csk=np.ascontiguousarray(np.concatenate(
                [np.tile(k_sin[b][:, 0::2], (1, HPG)),
                 np.tile(k_sin[b][:, 1::2], (1, HPG))], axis=1)),
        ))
    per_g = []
    for g in range(G):
        rows = slice(g * GD, (g + 1) * GD)
        per_g.append(dict(
            wqT=np.ascontiguousarray(Wq[rows, :][_PERM, :].T.astype(NP_BF16)),
            wkT=np.ascontiguousarray(Wk[rows, :][_PERM, :].T.astype(NP_BF16)),
            wvT=np.ascontiguousarray(Wv[rows, :].T.astype(NP_BF16)),
            woS=np.ascontiguousarray(Wo[:, rows].T.astype(NP_BF16)),
            ls=np.broadcast_to(ls_e[g * HPG:(g + 1) * HPG][None, :], (P, HPG)).copy(),
        ))

    qs_bc = (np.broadcast_to(np.tile(qns, HPG)[_PERM][None, :], (P, GD)).copy()
             if apply_qs else None)
    ks_bc = (np.broadcast_to(np.tile(kns, HPG)[_PERM][None, :], (P, GD)).copy()
             if apply_ks else None)

    in_maps = []
    for c in range(8):
        b, g = divmod(c, G)
        m = dict(expBT=expBT)
        m.update(per_b[b])
        m.update(per_g[g])
        if apply_qs:
            m['qscale'] = qs_bc
        if apply_ks:
            m['kscale'] = ks_bc
        in_maps.append(m)
    return in_maps, apply_qs, apply_ks


_CACHE = {}


def _get_nc(apply_qs, apply_ks):
    key = (apply_qs, apply_ks)
    if key not in _CACHE:
        _CACHE[key] = _build(apply_qs, apply_ks)
    return _CACHE[key]


def kernel(**inputs) -> np.ndarray:
    in_maps, apply_qs, apply_ks = _prepare(inputs)
    nc = _get_nc(apply_qs, apply_ks)
    res = run_bass_kernel_spmd(nc, in_maps, core_ids=list(range(8)))
    out = np.zeros((B, L, E), np.float32)
    for c in range(8):
        b = c // G
        out[b] += res.results[c]["out"].astype(np.float32)
    return out


# revision 58
# speedup vs baseline: 1.0848x; 1.0848x over previous
"""Multi-head scaled-cosine attention (B=2, L=2048, E=2048, H=16, D=128) on 8 trn2 cores.

Sharding: core c = (b, g) with b = batch (2), g = head-group of 4 heads (4 groups).
Each core computes its 4 heads' attention for its batch plus the partial output
projection; the host sums the 4 per-group partials per batch.

Precision: matmuls run in bf16 (PSUM accumulation is fp32). The Q/K RMS-norm
cancels exactly under the subsequent L2 normalization; the L2 reciprocal (and
logit scale) are folded into a diagonal matrix applied by the PE transpose
that produces Q^T/K^T. exp(bias - rowmax) is precomputed on the host and
folded in multiplicatively. Scores are built directly in [k, q] orientation so
softmax and attn@V need no on-chip transpose of the probability matrix. Q/K
head dims are host-permuted (evens|odds) so RoPE uses contiguous vector ops;
the permutation cancels inside q.k.

Softmax denominator: adjacent k-tiles' probability tiles are pair-summed on
DVE right behind the exp*bias mults, halving the columns the ones-matmul must
stream on the PE; the ones-matmuls run as a late burst (kt14/15 + loop end,
the last two k-tiles raw) so no PSUM bank is resident during the loop and the
Ln/exp(-ln) 1/den chain (and its activation-table loads) completes inside the
next chunk's exp-free warm window.

Scheduling: the first kv x-block is split across four engine DMA queues and
posted ahead of the weights, and the first V-projection runs as two l-half
chains, so the PE starts within a few us. Q^T/K^T transposes run 1-2 l-tiles
behind their norm chain (square+reduce on DVE so only the sqrt is scalar).
The last eight Q l-tiles' normalize tails are deferred into the first
attention q-chunk (their 8 sqrts batched into one scalar op, the vector tails
paced every other k-tile, transposes one k-tile behind). Every q-chunk's
first two k-tiles (scores+exp+bias-mult) are computed in the shadow of the
previous phase ("warm"), so the in-order PE queue crosses phase and chunk
boundaries without waiting on the scalar/DVE pipelines; PV runs 3+ k-tiles
behind scores, and outproj tiles of the previous chunk drain one per k-tile
as PE filler.
"""
import sys
sys.path.insert(0, '/opt/trn_rl_repo')
import math
import numpy as np
import ml_dtypes

import concourse.bacc as bacc
import concourse.mybir as mybir
import concourse.tile as tile
from concourse.bass_utils import run_bass_kernel_spmd

F32 = mybir.dt.float32
F32R = mybir.dt.float32r
BF16 = mybir.dt.bfloat16
NP_BF16 = ml_dtypes.bfloat16
ALU = mybir.AluOpType
AF = mybir.ActivationFunctionType

B, L, E, H, D = 2, 2048, 2048, 16, 128
G = 4                 # head groups
HPG = H // G          # heads per group = 4
GD = HPG * D          # 512, per-group projection width
P = 128               # partitions
NLT = L // P          # 16 l-tiles
NET = E // P          # 16 e-tiles (contraction)
NQC = L // 512        # 4 q-chunks
NKT = L // P          # 16 k-tiles
HD2 = GD // 2         # 256
LOGIT_SCALE_MAX = math.log(1.0 / 0.01)


def _build(apply_qs: bool, apply_ks: bool):
    nc = bacc.Bacc(None, target_bir_lowering=False)
    d = {}
    # x inputs in [p, l, e16] layout: per partition, per l, 16 contiguous
    # e-chunk entries -> one 4KB contiguous run per partition per l-block DMA.
    d['xqT'] = nc.dram_tensor("xqT", [P, L * NET], BF16, kind="ExternalInput")
    d['xkvT'] = nc.dram_tensor("xkvT", [P, L * NET], BF16, kind="ExternalInput")
    d['expBT'] = nc.dram_tensor("expBT", [L, L], BF16, kind="ExternalInput")
    d['wqT'] = nc.dram_tensor("wqT", [E, GD], BF16, kind="ExternalInput")
    d['wkT'] = nc.dram_tensor("wkT", [E, GD], BF16, kind="ExternalInput")
    d['wvT'] = nc.dram_tensor("wvT", [E, GD], BF16, kind="ExternalInput")
    d['woS'] = nc.dram_tensor("woS", [GD, E], BF16, kind="ExternalInput")
    d['csq'] = nc.dram_tensor("csq", [L, GD], F32, kind="ExternalInput")
    d['csk'] = nc.dram_tensor("csk", [L, GD], F32, kind="ExternalInput")
    d['ls'] = nc.dram_tensor("ls", [P, HPG], F32, kind="ExternalInput")
    if apply_qs:
        d['qscale'] = nc.dram_tensor("qscale", [P, GD], F32, kind="ExternalInput")
    if apply_ks:
        d['kscale'] = nc.dram_tensor("kscale", [P, GD], F32, kind="ExternalInput")
    out = nc.dram_tensor("out", [L, E], BF16, kind="ExternalOutput")

    with tile.TileContext(nc) as tc:
        with tc.tile_pool(name="persist", bufs=1) as persist:
            qT = persist.tile([P, HPG, L], BF16, tag="qT", name="qT")
            kT = persist.tile([P, HPG, L], BF16, tag="kT", name="kT")
            v_sb = persist.tile([P, NLT, GD], BF16, tag="v_sb")
            identb = persist.tile([P, P], BF16, tag="identb")
            identf = persist.tile([P, P], F32, tag="identf")
            nc.vector.memset(identf[:], 0.0)
            nc.gpsimd.affine_select(out=identf[:], in_=identf[:],
                                    compare_op=ALU.not_equal, fill=1.0, base=0,
                                    pattern=[[-1, P]], channel_multiplier=1)
            nc.vector.tensor_copy(identb[:], identf[:])
            ones_f = persist.tile([P, P], F32, tag="ones_f")
            nc.vector.memset(ones_f[:], 1.0)
            ones_r = persist.tile([P, P], F32R, tag="ones_r")
            nc.scalar.copy(ones_r[:], ones_f[:])
            ones_b = persist.tile([P, P], BF16, tag="ones_b")
            nc.vector.tensor_copy(ones_b[:], ones_f[:])
            ls_t = persist.tile([P, HPG], F32, tag="ls_t")

            wo_sb = persist.tile([P, HPG, E], BF16, tag="wo_sb")

            qs_t = ks_t = None
            if apply_qs:
                qs_t = persist.tile([P, GD], F32, tag="qs_t")
            if apply_ks:
                ks_t = persist.tile([P, GD], F32, tag="ks_t")

            q3_hold = persist.tile([P, 8, GD], BF16, tag="q3_hold")
            nrm_hold = persist.tile([P, 8 * HPG], F32, tag="nrm_hold")
            sq_hold = persist.tile([P, 8 * HPG], F32, tag="sq_hold")
            # first attention q-chunk's kt=0,1 probability tiles, computed in
            # the shadow of the Q-projection tail so the PE never idles at the
            # projection->attention boundary
            warm_pt = persist.tile([P, 2, 2048], BF16, tag="warm_pt")
            warm_eb = persist.tile([P, 2, 512], BF16, tag="warm_eb")

            from contextlib import ExitStack
            proj_ctx = ExitStack()
            sbp = proj_ctx.enter_context(tc.tile_pool(name="proj_sb", bufs=7))
            wpool = proj_ctx.enter_context(tc.tile_pool(name="proj_w", bufs=1))
            w_all = {}
            for wname in ('wvT', 'wkT', 'wqT'):
                w_all[wname] = wpool.tile([P, NET, GD], BF16, tag=wname,
                                          name=f"w_{wname}")
            nrm = proj_ctx.enter_context(tc.tile_pool(name="proj_nrm", bufs=6))
            psp = proj_ctx.enter_context(tc.tile_pool(name="proj_ps", bufs=4, space="PSUM"))
            pst = proj_ctx.enter_context(tc.tile_pool(name="proj_pst", bufs=2, space="PSUM"))

            # ---- head DMAs: first x blocks (queue-split) ahead of weights ----
            blks = {}

            def load_blk(key, lt, x_dram, split=1, engines=None):
                blk = sbp.tile([P, P, NET], BF16, tag="xblk", name=f"xblk_{key}_{lt}")
                ch = P // split
                engines = engines or [nc.sync] * split
                for s in range(split):
                    engines[s].dma_start(
                        blk[:, s * ch:(s + 1) * ch, :],
                        x_dram[:, (lt * P + s * ch) * NET:(lt * P + (s + 1) * ch) * NET]
                            .rearrange("p (l e) -> p l e", e=NET))
                blks[(key, lt)] = blk

            def load_w(wname, e0, e1):
                nc.sync.dma_start(
                    w_all[wname][:, e0:e1, :],
                    d[wname][e0 * P:e1 * P, :].rearrange("(e p) c -> p e c", p=P))

            nc.scalar.dma_start(w_all['wvT'][:, 0:1, :],
                                d['wvT'][0:P, :].rearrange("(e p) c -> p e c", p=P))
            load_blk('kv', 0, d['xkvT'], split=8,
                     engines=[nc.sync, nc.gpsimd, nc.scalar, nc.sync,
                              nc.gpsimd, nc.scalar, nc.sync, nc.gpsimd])
            load_blk('kv', 1, d['xkvT'], split=2,
                     engines=[nc.gpsimd, nc.sync])
            load_w('wvT', 1, 4)
            load_w('wvT', 4, 10)
            load_w('wvT', 10, 16)
            kv_loaded = 2
            for g4 in range(4):
                load_w('wkT', 4 * g4, 4 * g4 + 4)
            nc.sync.dma_start(ls_t[:], d['ls'][:])

            def qk_norm(lt, psum, cs_dram, scale_tile, use_ls, q3_out=None,
                        nrm_out=None, ve=None, ce=None):
                """RoPE + L2-normalize one l-tile; returns q3 [P, GD] bf16.
                With nrm_out set, stops after the sqrt (writing |q| there and
                the un-normalized rotated values to q3_out) so the normalize
                tail can be emitted later without blocking the scalar queue.
                ve picks the engine for the heavy elementwise ops (DVE by
                default; GpSimd for late held tiles to clear DVE backlog)."""
                ve = ve or nc.vector
                q1 = nrm.tile([P, GD], BF16, tag="q1")
                if ce is None:
                    nc.scalar.copy(q1[:], psum)
                else:
                    ce.tensor_copy(q1[:], psum)
                if scale_tile is not None:
                    nc.vector.tensor_mul(q1[:], q1[:], scale_tile[:])
                cst = nrm.tile([P, GD], F32, tag="cst")
                nc.sync.dma_start(cst[:], cs_dram[lt * P:(lt + 1) * P, :])
                ct, st = cst[:, :HD2], cst[:, HD2:]
                # per-head layout [evens(64) | odds(64)] (host-permuted weights)
                q1v = q1[:].rearrange("p (hh par dd) -> p hh par dd", hh=HPG, par=2)
                qe, qo = q1v[:, :, 0, :], q1v[:, :, 1, :]
                q2 = q3_out if q3_out is not None \
                    else nrm.tile([P, GD], BF16, tag="q2", name="q2")[:]
                q2v = q2.rearrange("p (hh par dd) -> p hh par dd", hh=HPG, par=2)
                re, ro = q2v[:, :, 0, :], q2v[:, :, 1, :]
                ctv = ct.rearrange("p (hh dd) -> p hh dd", hh=HPG)
                stv = st.rearrange("p (hh dd) -> p hh dd", hh=HPG)
                tmp = nrm.tile([P, HD2], BF16, tag="tmp")
                tv = tmp[:].rearrange("p (hh dd) -> p hh dd", hh=HPG)
                # evens: qe*c - qo*s ; odds: qo*c + qe*s
                ve.tensor_tensor(tv, qo, stv, ALU.mult)
                ve.tensor_tensor(re, qe, ctv, ALU.mult)
                ve.tensor_sub(re, re, tv)
                ve.tensor_tensor(tv, qe, stv, ALU.mult)
                ve.tensor_tensor(ro, qo, ctv, ALU.mult)
                ve.tensor_add(ro, ro, tv)
                # L2 norm over each head's (now contiguous) D slice; the
                # square+reduce stays off the scalar queue so only the sqrt
                # is scalar
                sqs = nrm.tile([P, GD], BF16, tag="sqs")
                ve.tensor_tensor(sqs[:], q2, q2, ALU.mult)
                if nrm_out is not None:
                    # held tile: stash the raw sum of squares; all 8 sqrts run
                    # as one batched scalar op inside the first q-chunk
                    nc.vector.tensor_reduce(
                        nrm_out, sqs[:].rearrange("p (hh dd) -> p hh dd", hh=HPG),
                        mybir.AxisListType.X, ALU.add)
                    return None
                acc = nrm.tile([P, HPG], F32, tag="acc")
                nc.vector.tensor_reduce(
                    acc[:], sqs[:].rearrange("p (hh dd) -> p hh dd", hh=HPG),
                    mybir.AxisListType.X, ALU.add)
                nrm_t = nrm.tile([P, HPG], F32, tag="nrm_t", name="nrm_t")[:]
                nc.scalar.activation(nrm_t, acc[:], AF.Sqrt)
                nc.vector.tensor_scalar_max(nrm_t, nrm_t, 1e-12)
                rcp = nrm.tile([P, HPG], F32, tag="rcp")
                nc.vector.reciprocal(rcp[:], nrm_t)
                if use_ls:
                    nc.vector.tensor_mul(rcp[:], rcp[:], ls_t[:])
                q3 = nrm.tile([P, GD], BF16, tag="q3", name="q3")[:]
                for h in range(HPG):
                    nc.vector.tensor_scalar_mul(q3[:, h * D:(h + 1) * D],
                                                q2[:, h * D:(h + 1) * D], rcp[:, h:h + 1])
                return q3

            pend_tr = []

            def emit_transposes(pend, pool):
                lt, q3, dstT = pend
                pt = pool.tile([P, GD], BF16, tag="pt" if pool is pst else "sh",
                               name=f"pt_{lt}")
                for h in range(HPG):
                    nc.tensor.matmul(pt[:, h * D:(h + 1) * D], q3[:, h * D:(h + 1) * D],
                                     identb[:], is_transpose=True)
                nc.any.tensor_copy(
                    dstT[:, :, lt * P:(lt + 1) * P],
                    pt[:].rearrange("p (h dd) -> p h dd", h=HPG))

            # merged V+K phase: one xkvT block load feeds both projections
            for lt in range(NLT):
                while kv_loaded <= lt + 5 and kv_loaded < NLT:
                    load_blk('kv', kv_loaded, d['xkvT'])
                    kv_loaded += 1
                if lt == 6:
                    for g4 in range(4):
                        load_w('wqT', 4 * g4, 4 * g4 + 4)
                    if apply_qs:
                        nc.sync.dma_start(qs_t[:], d['qscale'][:])
                    if apply_ks:
                        nc.sync.dma_start(ks_t[:], d['kscale'][:])
                if lt == 10:
                    nc.sync.dma_start(
                        wo_sb[:], d['woS'][:].rearrange("(h p) e -> p h e", p=P))
                blk = blks.pop(('kv', lt))
                psum_v = psp.tile([P, GD], F32, tag="psum", name=f"psumv_{lt}")
                if lt == 0:
                    for e in range(NET):
                        nc.tensor.matmul(psum_v[0:64, :], blk[:, 0:64, e],
                                         w_all['wvT'][:, e, :],
                                         start=(e == 0), stop=(e == NET - 1))
                    for e in range(NET):
                        nc.tensor.matmul(psum_v[64:128, :], blk[:, 64:128, e],
                                         w_all['wvT'][:, e, :],
                                         start=(e == 0), stop=(e == NET - 1))
                else:
                    for e in range(NET):
                        nc.tensor.matmul(psum_v[:], blk[:, :, e],
                                         w_all['wvT'][:, e, :],
                                         start=(e == 0), stop=(e == NET - 1))
                psum_k = psp.tile([P, GD], F32, tag="psum", name=f"psumk_{lt}")
                for e in range(NET):
                    nc.tensor.matmul(psum_k[:], blk[:, :, e], w_all['wkT'][:, e, :],
                                     start=(e == 0), stop=(e == NET - 1))
                while len(pend_tr) > 1:
                    emit_transposes(pend_tr.pop(0), pst)
                nc.scalar.copy(v_sb[:, lt, :], psum_v[:])
                q3 = qk_norm(lt, psum_k[:], d['csk'], ks_t, False)
                pend_tr.append((lt, q3, kT))

            def warm_phase(nqc, kt, ph, pool):
                """Emit one slice of the (nqc, kt<2) attention warmup: scores,
                exp and bias-mult run in the shadow of the previous phase so
                the epilogue's scalar table loads never gate them."""
                ptw = warm_pt[:, kt, :]
                if ph == 0:
                    nc.sync.dma_start(
                        warm_eb[:, kt, :],
                        d['expBT'][kt * P:(kt + 1) * P, nqc * 512:(nqc + 1) * 512])
                    hs = (0, 1)
                elif ph == 1:
                    hs = (2, 3)
                else:
                    hs = ()
                for h in hs:
                    s_ps = pool.tile([P, GD], F32,
                                     tag="s_w" if pool is pst else "sh",
                                     name=f"sw{nqc}_{kt}_{h}")
                    nc.tensor.matmul(s_ps[:], kT[:, h, kt * P:(kt + 1) * P],
                                     qT[:, h, nqc * 512:(nqc + 1) * 512],
                                     start=True, stop=True)
                    nc.scalar.activation(ptw[:, h * 512:(h + 1) * 512], s_ps[:],
                                         AF.Exp)
                if ph in (1, 2):
                    hpp = ph - 1
                    ebb = warm_eb[:, kt, :].rearrange(
                        "p (o q) -> p o q", o=1).broadcast_to([P, 2, 512])
                    half = ptw[:, hpp * 1024:(hpp + 1) * 1024].rearrange(
                        "p (i q) -> p i q", i=2)
                    nc.vector.tensor_tensor(half, half, ebb, ALU.mult)

            # process the l-tiles needed last (by the last q-chunk) FIRST so
            # the end-of-loop norm-chain drain only delays tiles nobody needs
            # soon; hold the last 8 tiles' normalize tails back as PE filler
            # for the first attention q-chunk
            q_order = list(range(12, 16)) + list(range(12))
            q_loaded = 2
            held_slots = []
            for i in range(2):
                load_blk('q', q_order[i], d['xqT'])
            for i, lt in enumerate(q_order):
                while q_loaded <= i + 5 and q_loaded < NLT:
                    load_blk('q', q_order[q_loaded], d['xqT'])
                    q_loaded += 1
                blk = blks.pop(('q', lt))
                psum_q = psp.tile([P, GD], F32, tag="psum", name=f"psumq_{lt}")
                for e in range(NET):
                    nc.tensor.matmul(psum_q[:], blk[:, :, e], w_all['wqT'][:, e, :],
                                     start=(e == 0), stop=(e == NET - 1))
                while len(pend_tr) > 2:
                    emit_transposes(pend_tr.pop(0), pst)
                if i in (9, 10) and pend_tr:
                    # drain the remaining deferred transposes early so their
                    # PSUM-freeing copies don't gate the first attention scores
                    emit_transposes(pend_tr.pop(0), pst)
                if i >= 11:
                    for kt_w, ph_w in ((0, i - 11), (1, i - 13)):
                        if 0 <= ph_w <= 2:
                            warm_phase(0, kt_w, ph_w, pst)
                if i >= 8:
                    j = i - 8
                    qk_norm(lt, psum_q[:], d['csq'], qs_t, True,
                            q3_out=q3_hold[:, j, :],
                            nrm_out=nrm_hold[:, j * HPG:(j + 1) * HPG])
                    held_slots.append((lt, j))
                else:
                    q3 = qk_norm(lt, psum_q[:], d['csq'], qs_t, True)
                    pend_tr.append((lt, q3, qT))
            while pend_tr:
                emit_transposes(pend_tr.pop(0), pst)
            proj_ctx.close()

            # attention per q-chunk
            att_ctx = ExitStack()
            asb = att_ctx.enter_context(tc.tile_pool(name="att_sb", bufs=3))
            atp = att_ctx.enter_context(tc.tile_pool(name="att_at", bufs=1))
            aop = att_ctx.enter_context(tc.tile_pool(name="att_o", bufs=4))
            ps_pv = att_ctx.enter_context(tc.tile_pool(name="ps_pv", bufs=1, space="PSUM"))
            ps_sh = att_ctx.enter_context(tc.tile_pool(name="ps_sh", bufs=4, space="PSUM"))

            out_tasks = []
            pend_norm = None
            held_ready = []

            def emit_out_task(task, final=False):
                oqc, attn, lsub, ec = task
                o_ps = ps_sh.tile([P, 512], F32, tag="sh", name=f"o{oqc}_{lsub}_{ec}")
                for h in range(HPG):
                    nc.tensor.matmul(o_ps[:], attn[h][:, lsub * P:(lsub + 1) * P],
                                     wo_sb[:, h, ec * 512:(ec + 1) * 512],
                                     start=(h == 0), stop=(h == HPG - 1))
                o_sb = aop.tile([P, 512], BF16, tag="o_sb", name=f"ob{oqc}_{lsub}_{ec}")
                if final and (lsub + ec) % 2 == 1:
                    nc.scalar.copy(o_sb[:], o_ps[:])
                else:
                    nc.vector.tensor_copy(o_sb[:], o_ps[:])
                nc.gpsimd.dma_start(
                    out[oqc * 512 + lsub * P: oqc * 512 + (lsub + 1) * P,
                        ec * 512:(ec + 1) * 512], o_sb[:])

            def emit_held_dve(slot):
                """Deferred (vector-only) normalize tail for a held Q l-tile."""
                hlt, j = slot
                nv = sq_hold[:, j * HPG:(j + 1) * HPG]
                rcp = asb.tile([P, HPG], F32, tag="hrcp", name=f"hrcp{j}", bufs=2)
                nc.vector.tensor_scalar_max(rcp[:], nv, 1e-12)
                nc.vector.reciprocal(rcp[:], rcp[:])
                nc.vector.tensor_mul(rcp[:], rcp[:], ls_t[:])
                q3 = q3_hold[:, j, :]
                for h in range(HPG):
                    nc.vector.tensor_scalar_mul(q3[:, h * D:(h + 1) * D],
                                                q3[:, h * D:(h + 1) * D],
                                                rcp[:, h:h + 1])
                return (hlt, q3)

            def emit_den_group(den_t, dg, g):
                for h in range(HPG):
                    nc.tensor.matmul(den_t[32 * h:32 * h + 32, :],
                                     ones_b[:, 0:32],
                                     dg[:, h * 512:(h + 1) * 512],
                                     start=(g == 0), stop=(g == NKT // 2 - 1),
                                     tile_position=(0, 32 * h))

            def emit_epilogue(pend):
                """Ln + exp(-x) chain for a q-chunk's 1/den; the scalar table
                loads land in the exp-free warm window of the next chunk."""
                pqc = pend['qc']
                lnd = asb.tile([P, 512], F32, tag="lnd", name=f"lnd{pqc}", bufs=2)
                nc.scalar.activation(lnd[:], pend['den'][:], AF.Ln)
                rcp32s = []
                for i in range(2):
                    r32 = asb.tile([64, 512], F32R, tag=f"rcp32{i}",
                                   name=f"rcp32{pqc}_{i}", bufs=2)
                    nc.scalar.activation(r32[:], lnd[64 * i:64 * i + 64, :],
                                         AF.Exp, scale=-1.0)
                    rcp32s.append(r32)
                pend['rcp32s'] = rcp32s

            def emit_norm(pend):
                """Broadcast 1/den across partitions and normalize pv -> attn;
                returns the outproj tile tasks for this q-chunk."""
                nqc, npvc, rcp32s = pend['qc'], pend['pvc'], pend['rcp32s']
                attn = [atp.tile([P, 512], BF16, tag=f"at{h}", name=f"at{nqc}_{h}",
                                 bufs=2) for h in range(HPG)]
                for h in range(HPG):
                    b_ps = ps_sh.tile([P, 512], F32, tag="sh", name=f"b{nqc}_{h}")
                    off = 32 * (h % 2)
                    nc.tensor.matmul(b_ps[:], ones_r[off:off + 1, :],
                                     rcp32s[h // 2][off:off + 1, :],
                                     start=True, stop=True)
                    rcpb = asb.tile([P, 512], BF16, tag="rcpb", name=f"rb{nqc}_{h}")
                    nc.vector.tensor_copy(rcpb[:], b_ps[:])
                    nc.vector.tensor_mul(attn[h][:], npvc[h], rcpb[:])
                return [(nqc, attn, lsub, ec)
                        for lsub in range(4) for ec in range(4)]

            for qc in range(NQC):
                qsl = slice(qc * 512, (qc + 1) * 512)
                last = qc == NQC - 1
                pv = [ps_pv.tile([P, 512], F32, tag=f"pv{h}", name=f"pv{qc}_{h}")
                      for h in range(HPG)]

                eb_tiles = {}

                def post_eb(kt):
                    eb = asb.tile([P, 512], BF16, tag="eb", name=f"eb{qc}_{kt}")
                    nc.sync.dma_start(eb[:], d['expBT'][kt * P:(kt + 1) * P, qsl])
                    eb_tiles[kt] = eb

                def stage1(kt):
                    # bias tile for kt+1 posted now so its transfer hides
                    # under this k-tile's compute
                    if kt + 1 < NKT:
                        post_eb(kt + 1)
                    eb = eb_tiles.pop(kt)
                    p_t = asb.tile([P, 2048], BF16, tag="p_t", name=f"pt{qc}_{kt}", bufs=5)
                    ebb = eb[:].rearrange("p (o q) -> p o q", o=1).broadcast_to([P, 2, 512])
                    return p_t, ebb

                post_eb(2)

                def score_head(kt, p_t, h):
                    s_ps = ps_sh.tile([P, 512], F32, tag="sh", name=f"sp{qc}_{kt}_{h}")
                    nc.tensor.matmul(s_ps[:], kT[:, h, kt * P:(kt + 1) * P],
                                     qT[:, h, qsl], start=True, stop=True)
                    nc.scalar.activation(p_t[:, h * 512:(h + 1) * 512], s_ps[:], AF.Exp)

                def mult_half(p_t, ebb, hpp):
                    half = p_t[:, hpp * 1024:(hpp + 1) * 1024].rearrange(
                        "p (i q) -> p i q", i=2)
                    nc.vector.tensor_tensor(half, half, ebb, ALU.mult)

                def stage2_pv(kt, p_t):
                    for h in range(HPG):
                        nc.tensor.matmul(pv[h][:], v_sb[:, kt, h * D:(h + 1) * D],
                                         p_t[:, h * 512:(h + 1) * 512],
                                         start=(kt == 0), stop=(kt == NKT - 1))

                # den pre-reduction: pairs of k-tiles are summed on DVE right
                # behind the exp*bias mults; the ones-matmuls run as a burst
                # in the next chunk's warm window (no resident PSUM bank, and
                # the Ln/Exp table thrash lands where the scalar queue is idle)
                dacc = {}
                grp_first = None
                den_t = None

                s2q = []
                for kt in range(NKT):
                    warm = kt < 2
                    if warm:
                        # scores/exp/mult for these tiles ran in the shadow of
                        # the previous phase
                        p_t = warm_pt[:, kt, :]
                    else:
                        p_t, ebb = stage1(kt)
                        score_head(kt, p_t, 0)
                        score_head(kt, p_t, 1)
                        mult_half(p_t, ebb, 0)
                        score_head(kt, p_t, 2)
                    # PE fillers go BEFORE the stage2 pop so the in-order PE
                    # queue has ready work while exp/mult of prev completes
                    if pend_norm is not None and kt == 5:
                        out_tasks.extend(emit_norm(pend_norm))
                        pend_norm = None
                    if kt == 14:
                        # den burst late in the loop: no resident PSUM bank
                        # before this, and the Ln/Exp chain finishes before
                        # the next chunk's scores rotate into these banks
                        den_t = ps_sh.tile([P, 512], F32, tag="sh",
                                           name=f"den{qc}")
                        for g in range(5):
                            emit_den_group(den_t, dacc[g], g)
                    elif kt == 15:
                        emit_den_group(den_t, dacc[5], 5)
                        emit_den_group(den_t, dacc[6], 6)
                        for h in range(HPG):
                            nc.tensor.matmul(
                                den_t[32 * h:32 * h + 32, :],
                                ones_b[:, 0:32],
                                s2q[-1][1][:, h * 512:(h + 1) * 512],
                                start=False, stop=False,
                                tile_position=(0, 32 * h))
                    if held_ready and kt >= 5 and kt % 2 == 1:
                        emit_transposes(held_ready.pop(0) + (qT,), ps_sh)
                    for _ in range(2 if kt < 2 else 1):
                        if out_tasks:
                            emit_out_task(out_tasks.pop(0))
                    # PV runs 2+ k-tiles behind scores so the cross-engine
                    # score->exp->mult chain latency never stalls the PE; qc0
                    # lags more so the projection-phase queue drain hides
                    lag = 4 if (qc == 0 and kt < 8) else 3
                    if len(s2q) >= lag:
                        stage2_pv(*s2q.pop(0))
                    if not warm:
                        score_head(kt, p_t, 3)
                        mult_half(p_t, ebb, 1)
                    if held_slots and kt == 2:
                        nc.scalar.activation(sq_hold[:], nrm_hold[:], AF.Sqrt)
                    if held_slots and kt >= 4 and kt % 2 == 0:
                        held_ready.append(emit_held_dve(held_slots.pop(0)))
                    if qc < NQC - 1 and kt >= 12:
                        for kt_w, ph_w in ((0, kt - 12), (1, kt - 13)):
                            if 0 <= ph_w <= 2:
                                warm_phase(qc + 1, kt_w, ph_w, ps_sh)
                    g, r = divmod(kt, 2)
                    if kt >= 14:
                        pass  # raw den matmuls, no pair-add needed
                    elif r == 0:
                        grp_first = p_t
                    else:
                        dg = asb.tile([P, 2048], BF16, tag="dacc",
                                      name=f"dacc{qc}_{g}", bufs=8)
                        dacc[g] = dg
                        nc.vector.tensor_add(dg[:], grp_first[:], p_t[:])
                    s2q.append((kt, p_t))
                for item in s2q:
                    stage2_pv(*item)
                for h in range(HPG):
                    nc.tensor.matmul(den_t[32 * h:32 * h + 32, :],
                                     ones_b[:, 0:32],
                                     s2q[-1][1][:, h * 512:(h + 1) * 512],
                                     start=False, stop=True,
                                     tile_position=(0, 32 * h))
                pend = {'qc': qc, 'den': den_t}
                emit_epilogue(pend)
                if not last:
                    pvc = []
                    for h in range(HPG):
                        c = asb.tile([P, 512], BF16, tag=f"pvc{h}",
                                     name=f"pvc{qc}_{h}", bufs=2)
                        nc.vector.tensor_copy(c[:], pv[h][:])
                        pvc.append(c[:])
                    pend['pvc'] = pvc
                    pend_norm = pend
                else:
                    pend['pvc'] = [pv[h][:] for h in range(HPG)]
                    # drain carried tasks first: they fill the PE while the
                    # Ln/Exp chain and table loads run
                    while out_tasks:
                        emit_out_task(out_tasks.pop(0))
                    out_tasks.extend(emit_norm(pend))
            while out_tasks:
                emit_out_task(out_tasks.pop(0), final=True)
            att_ctx.close()
    nc.compile()
    return nc


# head-dim permutation: within each head, evens first then odds
_PERM = np.empty(GD, np.int64)
for _i in range(GD):
    _h, _j = divmod(_i, D)
    _par, _dd = divmod(_j, D // 2)
    _PERM[_i] = _h * D + 2 * _dd + _par


def _x_relayout(x):
    # [L, E] f32 -> [P, L*NET] bf16 with column index l*NET + e  (4KB runs)
    xt = np.ascontiguousarray(x.T.astype(NP_BF16))        # [E, L]
    xr = xt.reshape(NET, P, L).transpose(1, 2, 0)          # [P, L, NET]
    return np.ascontiguousarray(xr.reshape(P, L * NET))


def _prepare(inputs):
    f32 = np.float32
    inputs_q = np.asarray(inputs["inputs_q"], f32)
    inputs_kv = np.asarray(inputs["inputs_kv"], f32)
    bias = np.asarray(inputs["bias"], f32).reshape(L, L)
    q_sin = np.asarray(inputs["q_sinusoids"], f32)
    k_sin = np.asarray(inputs["k_sinusoids"], f32)
    Wq = np.asarray(inputs["Wq"], f32)
    Wk = np.asarray(inputs["Wk"], f32)
    Wv = np.asarray(inputs["Wv"], f32)
    Wo = np.asarray(inputs["Wo"], f32)
    qns = np.asarray(inputs["q_norm_scale"], f32)
    kns = np.asarray(inputs["k_norm_scale"], f32)
    ls = np.asarray(inputs["logit_scale"], f32)

    apply_qs = not np.all(qns == 1.0)
    apply_ks = not np.all(kns == 1.0)

    bm = bias.max(axis=1, keepdims=True)
    expBT = np.ascontiguousarray(np.exp((bias - bm).T).astype(NP_BF16))
    ls_e = np.exp(np.minimum(ls, LOGIT_SCALE_MAX)).astype(f32)

    per_b = []
    for b in range(B):
        per_b.append(dict(
            xqT=_x_relayout(inputs_q[b]),
            xkvT=_x_relayout(inputs_kv[b]),
            csq=np.ascontiguousarray(np.concatenate(
                [np.tile(q_sin[b][:, 0::2], (1, HPG)),
                 np.tile(q_sin[b][:, 1::2], (1, HPG))], axis=1)),
            csk=np.ascontiguousarray(np.concatenate(
                [np.tile(k_sin[b][:, 0::2], (1, HPG)),
                 np.tile(k_sin[b][:, 1::2], (1, HPG))], axis=1)),
        ))
    per_g = []
    for g in range(G):
        rows = slice(g * GD, (g + 1) * GD)
        per_g.append(dict(
            wqT=np.ascontiguousarray(Wq[rows, :][_PERM, :].T.astype(NP_BF16)),
            wkT=np.ascontiguousarray(Wk[rows, :][_PERM, :].T.astype(NP_BF16)),
            wvT=np.ascontiguousarray(Wv[rows, :].T.astype(NP_BF16)),
            woS=np.ascontiguousarray(Wo[:, rows].T.astype(NP_BF16)),
            ls=np.broadcast_to(ls_e[g * HPG:(g + 1) * HPG][None, :], (P, HPG)).copy(),
        ))

    qs_bc = (np.broadcast_to(np.tile(qns, HPG)[_PERM][None, :], (P, GD)).copy()
             if apply_qs else None)
    ks_bc = (np.broadcast_to(np.tile(kns, HPG)[_PERM][None, :], (P, GD)).copy()
             if apply_ks else None)

    in_maps = []
    for c in range(8):
        b, g = divmod(c, G)
        m = dict(expBT=expBT)
        m.update(per_b[b])
        m.update(per_g[g])
        if apply_qs:
            m['qscale'] = qs_bc
        if apply_ks:
            m['kscale'] = ks_bc
        in_maps.append(m)
    return in_maps, apply_qs, apply_ks


_CACHE = {}


def _get_nc(apply_qs, apply_ks):
    key = (apply_qs, apply_ks)
    if key not in _CACHE:
        _CACHE[key] = _build(apply_qs, apply_ks)
    return _CACHE[key]


def kernel(**inputs) -> np.ndarray:
    in_maps, apply_qs, apply_ks = _prepare(inputs)
    nc = _get_nc(apply_qs, apply_ks)
    res = run_bass_kernel_spmd(nc, in_maps, core_ids=list(range(8)))
    out = np.zeros((B, L, E), np.float32)
    for c in range(8):
        b = c // G
        out[b] += res.results[c]["out"].astype(np.float32)
    return out


# revision 60
# speedup vs baseline: 1.0893x; 1.0041x over previous
"""Multi-head scaled-cosine attention (B=2, L=2048, E=2048, H=16, D=128) on 8 trn2 cores.

Sharding: core c = (b, g) with b = batch (2), g = head-group of 4 heads (4 groups).
Each core computes its 4 heads' attention for its batch plus the partial output
projection; the host sums the 4 per-group partials per batch.

Precision: matmuls run in bf16 (PSUM accumulation is fp32). The Q/K RMS-norm
cancels exactly under the subsequent L2 normalization; the L2 reciprocal (and
logit scale) are folded into a diagonal matrix applied by the PE transpose
that produces Q^T/K^T. exp(bias - rowmax) is precomputed on the host and
folded in multiplicatively. Scores are built directly in [k, q] orientation so
softmax and attn@V need no on-chip transpose of the probability matrix. Q/K
head dims are host-permuted (evens|odds) so RoPE uses contiguous vector ops;
the permutation cancels inside q.k.

Softmax denominator: adjacent k-tiles' probability tiles are pair-summed on
DVE right behind the exp*bias mults, halving the columns the ones-matmul must
stream on the PE; the ones-matmuls run as a late burst (kt14/15 + loop end,
the last two k-tiles raw) so no PSUM bank is resident during the loop and the
Ln/exp(-ln) 1/den chain (and its activation-table loads) completes inside the
next chunk's exp-free warm window.

Scheduling: the first kv x-block is split across four engine DMA queues and
posted ahead of the weights, and the first V-projection runs as two l-half
chains, so the PE starts within a few us. Q^T/K^T transposes run 1-2 l-tiles
behind their norm chain (square+reduce on DVE so only the sqrt is scalar).
The last eight Q l-tiles' normalize tails are deferred into the first
attention q-chunk (their 8 sqrts batched into one scalar op, the vector tails
paced every other k-tile, transposes one k-tile behind). Every q-chunk's
first two k-tiles (scores+exp+bias-mult) are computed in the shadow of the
previous phase ("warm"), so the in-order PE queue crosses phase and chunk
boundaries without waiting on the scalar/DVE pipelines; PV runs 3+ k-tiles
behind scores, and outproj tiles of the previous chunk drain one per k-tile
as PE filler.
"""
import sys
sys.path.insert(0, '/opt/trn_rl_repo')
import math
import numpy as np
import ml_dtypes

import concourse.bacc as bacc
import concourse.mybir as mybir
import concourse.tile as tile
from concourse.bass_utils import run_bass_kernel_spmd

F32 = mybir.dt.float32
F32R = mybir.dt.float32r
BF16 = mybir.dt.bfloat16
NP_BF16 = ml_dtypes.bfloat16
ALU = mybir.AluOpType
AF = mybir.ActivationFunctionType

B, L, E, H, D = 2, 2048, 2048, 16, 128
G = 4                 # head groups
HPG = H // G          # heads per group = 4
GD = HPG * D          # 512, per-group projection width
P = 128               # partitions
NLT = L // P          # 16 l-tiles
NET = E // P          # 16 e-tiles (contraction)
NQC = L // 512        # 4 q-chunks
NKT = L // P          # 16 k-tiles
HD2 = GD // 2         # 256
LOGIT_SCALE_MAX = math.log(1.0 / 0.01)


def _build(apply_qs: bool, apply_ks: bool):
    nc = bacc.Bacc(None, target_bir_lowering=False)
    d = {}
    # x inputs in [p, l, e16] layout: per partition, per l, 16 contiguous
    # e-chunk entries -> one 4KB contiguous run per partition per l-block DMA.
    d['xqT'] = nc.dram_tensor("xqT", [P, L * NET], BF16, kind="ExternalInput")
    d['xkvT'] = nc.dram_tensor("xkvT", [P, L * NET], BF16, kind="ExternalInput")
    d['expBT'] = nc.dram_tensor("expBT", [L, L], BF16, kind="ExternalInput")
    d['wqT'] = nc.dram_tensor("wqT", [E, GD], BF16, kind="ExternalInput")
    d['wkT'] = nc.dram_tensor("wkT", [E, GD], BF16, kind="ExternalInput")
    d['wvT'] = nc.dram_tensor("wvT", [E, GD], BF16, kind="ExternalInput")
    d['woS'] = nc.dram_tensor("woS", [GD, E], BF16, kind="ExternalInput")
    d['csq'] = nc.dram_tensor("csq", [L, GD], F32, kind="ExternalInput")
    d['csk'] = nc.dram_tensor("csk", [L, GD], F32, kind="ExternalInput")
    d['ls'] = nc.dram_tensor("ls", [P, HPG], F32, kind="ExternalInput")
    if apply_qs:
        d['qscale'] = nc.dram_tensor("qscale", [P, GD], F32, kind="ExternalInput")
    if apply_ks:
        d['kscale'] = nc.dram_tensor("kscale", [P, GD], F32, kind="ExternalInput")
    out = nc.dram_tensor("out", [L, E], BF16, kind="ExternalOutput")

    with tile.TileContext(nc) as tc:
        with tc.tile_pool(name="persist", bufs=1) as persist:
            qT = persist.tile([P, HPG, L], BF16, tag="qT", name="qT")
            kT = persist.tile([P, HPG, L], BF16, tag="kT", name="kT")
            v_sb = persist.tile([P, NLT, GD], BF16, tag="v_sb")
            identb = persist.tile([P, P], BF16, tag="identb")
            identf = persist.tile([P, P], F32, tag="identf")
            nc.vector.memset(identf[:], 0.0)
            nc.gpsimd.affine_select(out=identf[:], in_=identf[:],
                                    compare_op=ALU.not_equal, fill=1.0, base=0,
                                    pattern=[[-1, P]], channel_multiplier=1)
            nc.vector.tensor_copy(identb[:], identf[:])
            ones_f = persist.tile([P, P], F32, tag="ones_f")
            nc.vector.memset(ones_f[:], 1.0)
            ones_r = persist.tile([P, P], F32R, tag="ones_r")
            nc.scalar.copy(ones_r[:], ones_f[:])
            ones_b = persist.tile([P, P], BF16, tag="ones_b")
            nc.vector.tensor_copy(ones_b[:], ones_f[:])
            ls_t = persist.tile([P, HPG], F32, tag="ls_t")

            wo_sb = persist.tile([P, HPG, E], BF16, tag="wo_sb")

            qs_t = ks_t = None
            if apply_qs:
                qs_t = persist.tile([P, GD], F32, tag="qs_t")
            if apply_ks:
                ks_t = persist.tile([P, GD], F32, tag="ks_t")

            q3_hold = persist.tile([P, 8, GD], BF16, tag="q3_hold")
            nrm_hold = persist.tile([P, 8 * HPG], F32, tag="nrm_hold")
            sq_hold = persist.tile([P, 8 * HPG], F32, tag="sq_hold")
            # first attention q-chunk's kt=0,1 probability tiles, computed in
            # the shadow of the Q-projection tail so the PE never idles at the
            # projection->attention boundary
            warm_pt = persist.tile([P, 2, 2048], BF16, tag="warm_pt")
            warm_eb = persist.tile([P, 2, 512], BF16, tag="warm_eb")

            from contextlib import ExitStack
            proj_ctx = ExitStack()
            sbp = proj_ctx.enter_context(tc.tile_pool(name="proj_sb", bufs=7))
            wpool = proj_ctx.enter_context(tc.tile_pool(name="proj_w", bufs=1))
            w_all = {}
            for wname in ('wvT', 'wkT', 'wqT'):
                w_all[wname] = wpool.tile([P, NET, GD], BF16, tag=wname,
                                          name=f"w_{wname}")
            nrm = proj_ctx.enter_context(tc.tile_pool(name="proj_nrm", bufs=6))
            psp = proj_ctx.enter_context(tc.tile_pool(name="proj_ps", bufs=4, space="PSUM"))
            pst = proj_ctx.enter_context(tc.tile_pool(name="proj_pst", bufs=2, space="PSUM"))

            # ---- head DMAs: first x blocks (queue-split) ahead of weights ----
            blks = {}

            def load_blk(key, lt, x_dram, split=1, engines=None):
                blk = sbp.tile([P, P, NET], BF16, tag="xblk", name=f"xblk_{key}_{lt}")
                ch = P // split
                engines = engines or [nc.sync] * split
                for s in range(split):
                    engines[s].dma_start(
                        blk[:, s * ch:(s + 1) * ch, :],
                        x_dram[:, (lt * P + s * ch) * NET:(lt * P + (s + 1) * ch) * NET]
                            .rearrange("p (l e) -> p l e", e=NET))
                blks[(key, lt)] = blk

            def load_w(wname, e0, e1):
                nc.sync.dma_start(
                    w_all[wname][:, e0:e1, :],
                    d[wname][e0 * P:e1 * P, :].rearrange("(e p) c -> p e c", p=P))

            nc.scalar.dma_start(w_all['wvT'][:, 0:1, :],
                                d['wvT'][0:P, :].rearrange("(e p) c -> p e c", p=P))
            load_blk('kv', 0, d['xkvT'], split=8,
                     engines=[nc.sync, nc.gpsimd, nc.scalar, nc.sync,
                              nc.gpsimd, nc.scalar, nc.sync, nc.gpsimd])
            load_blk('kv', 1, d['xkvT'], split=2,
                     engines=[nc.gpsimd, nc.sync])
            load_w('wvT', 1, 4)
            load_w('wvT', 4, 10)
            load_w('wvT', 10, 16)
            kv_loaded = 2
            for g4 in range(4):
                load_w('wkT', 4 * g4, 4 * g4 + 4)
            nc.sync.dma_start(ls_t[:], d['ls'][:])

            def qk_norm(lt, psum, cs_dram, scale_tile, use_ls, q3_out=None,
                        nrm_out=None, ve=None, ce=None):
                """RoPE + L2-normalize one l-tile; returns q3 [P, GD] bf16.
                With nrm_out set, stops after the sqrt (writing |q| there and
                the un-normalized rotated values to q3_out) so the normalize
                tail can be emitted later without blocking the scalar queue.
                ve picks the engine for the heavy elementwise ops (DVE by
                default; GpSimd for late held tiles to clear DVE backlog)."""
                ve = ve or nc.vector
                q1 = nrm.tile([P, GD], BF16, tag="q1")
                if ce is None:
                    nc.scalar.copy(q1[:], psum)
                else:
                    ce.tensor_copy(q1[:], psum)
                if scale_tile is not None:
                    nc.vector.tensor_mul(q1[:], q1[:], scale_tile[:])
                cst = nrm.tile([P, GD], F32, tag="cst")
                nc.sync.dma_start(cst[:], cs_dram[lt * P:(lt + 1) * P, :])
                ct, st = cst[:, :HD2], cst[:, HD2:]
                # per-head layout [evens(64) | odds(64)] (host-permuted weights)
                q1v = q1[:].rearrange("p (hh par dd) -> p hh par dd", hh=HPG, par=2)
                qe, qo = q1v[:, :, 0, :], q1v[:, :, 1, :]
                q2 = q3_out if q3_out is not None \
                    else nrm.tile([P, GD], BF16, tag="q2", name="q2")[:]
                q2v = q2.rearrange("p (hh par dd) -> p hh par dd", hh=HPG, par=2)
                re, ro = q2v[:, :, 0, :], q2v[:, :, 1, :]
                ctv = ct.rearrange("p (hh dd) -> p hh dd", hh=HPG)
                stv = st.rearrange("p (hh dd) -> p hh dd", hh=HPG)
                tmp = nrm.tile([P, HD2], BF16, tag="tmp")
                tv = tmp[:].rearrange("p (hh dd) -> p hh dd", hh=HPG)
                # evens: qe*c - qo*s ; odds: qo*c + qe*s
                ve.tensor_tensor(tv, qo, stv, ALU.mult)
                ve.tensor_tensor(re, qe, ctv, ALU.mult)
                ve.tensor_sub(re, re, tv)
                ve.tensor_tensor(tv, qe, stv, ALU.mult)
                ve.tensor_tensor(ro, qo, ctv, ALU.mult)
                ve.tensor_add(ro, ro, tv)
                # L2 norm over each head's (now contiguous) D slice; the
                # square+reduce stays off the scalar queue so only the sqrt
                # is scalar
                sqs = nrm.tile([P, GD], BF16, tag="sqs")
                ve.tensor_tensor(sqs[:], q2, q2, ALU.mult)
                if nrm_out is not None:
                    # held tile: stash the raw sum of squares; all 8 sqrts run
                    # as one batched scalar op inside the first q-chunk
                    nc.vector.tensor_reduce(
                        nrm_out, sqs[:].rearrange("p (hh dd) -> p hh dd", hh=HPG),
                        mybir.AxisListType.X, ALU.add)
                    return None
                acc = nrm.tile([P, HPG], F32, tag="acc")
                nc.vector.tensor_reduce(
                    acc[:], sqs[:].rearrange("p (hh dd) -> p hh dd", hh=HPG),
                    mybir.AxisListType.X, ALU.add)
                nrm_t = nrm.tile([P, HPG], F32, tag="nrm_t", name="nrm_t")[:]
                nc.scalar.activation(nrm_t, acc[:], AF.Sqrt)
                nc.vector.tensor_scalar_max(nrm_t, nrm_t, 1e-12)
                rcp = nrm.tile([P, HPG], F32, tag="rcp")
                nc.vector.reciprocal(rcp[:], nrm_t)
                if use_ls:
                    nc.vector.tensor_mul(rcp[:], rcp[:], ls_t[:])
                q3 = nrm.tile([P, GD], BF16, tag="q3", name="q3")[:]
                for h in range(HPG):
                    nc.vector.tensor_scalar_mul(q3[:, h * D:(h + 1) * D],
                                                q2[:, h * D:(h + 1) * D], rcp[:, h:h + 1])
                return q3

            pend_tr = []

            def emit_transposes(pend, pool):
                lt, q3, dstT = pend
                pt = pool.tile([P, GD], BF16, tag="pt" if pool is pst else "sh",
                               name=f"pt_{lt}")
                for h in range(HPG):
                    nc.tensor.matmul(pt[:, h * D:(h + 1) * D], q3[:, h * D:(h + 1) * D],
                                     identb[:], is_transpose=True)
                nc.any.tensor_copy(
                    dstT[:, :, lt * P:(lt + 1) * P],
                    pt[:].rearrange("p (h dd) -> p h dd", h=HPG))

            # merged V+K phase: one xkvT block load feeds both projections
            for lt in range(NLT):
                while kv_loaded <= lt + 5 and kv_loaded < NLT:
                    load_blk('kv', kv_loaded, d['xkvT'])
                    kv_loaded += 1
                if lt == 6:
                    for g4 in range(4):
                        load_w('wqT', 4 * g4, 4 * g4 + 4)
                    if apply_qs:
                        nc.sync.dma_start(qs_t[:], d['qscale'][:])
                    if apply_ks:
                        nc.sync.dma_start(ks_t[:], d['kscale'][:])
                if lt == 10:
                    nc.sync.dma_start(
                        wo_sb[:], d['woS'][:].rearrange("(h p) e -> p h e", p=P))
                blk = blks.pop(('kv', lt))
                psum_v = psp.tile([P, GD], F32, tag="psum", name=f"psumv_{lt}")
                if lt == 0:
                    for e in range(NET):
                        nc.tensor.matmul(psum_v[0:64, :], blk[:, 0:64, e],
                                         w_all['wvT'][:, e, :],
                                         start=(e == 0), stop=(e == NET - 1))
                    for e in range(NET):
                        nc.tensor.matmul(psum_v[64:128, :], blk[:, 64:128, e],
                                         w_all['wvT'][:, e, :],
                                         start=(e == 0), stop=(e == NET - 1))
                else:
                    for e in range(NET):
                        nc.tensor.matmul(psum_v[:], blk[:, :, e],
                                         w_all['wvT'][:, e, :],
                                         start=(e == 0), stop=(e == NET - 1))
                psum_k = psp.tile([P, GD], F32, tag="psum", name=f"psumk_{lt}")
                for e in range(NET):
                    nc.tensor.matmul(psum_k[:], blk[:, :, e], w_all['wkT'][:, e, :],
                                     start=(e == 0), stop=(e == NET - 1))
                while len(pend_tr) > 1:
                    emit_transposes(pend_tr.pop(0), pst)
                nc.scalar.copy(v_sb[:, lt, :], psum_v[:])
                q3 = qk_norm(lt, psum_k[:], d['csk'], ks_t, False)
                pend_tr.append((lt, q3, kT))

            def warm_phase(nqc, kt, ph, pool):
                """Emit one slice of the (nqc, kt<2) attention warmup: scores,
                exp and bias-mult run in the shadow of the previous phase so
                the epilogue's scalar table loads never gate them."""
                ptw = warm_pt[:, kt, :]
                if ph == 0:
                    nc.sync.dma_start(
                        warm_eb[:, kt, :],
                        d['expBT'][kt * P:(kt + 1) * P, nqc * 512:(nqc + 1) * 512])
                    hs = (0, 1)
                elif ph == 1:
                    hs = (2, 3)
                else:
                    hs = ()
                for h in hs:
                    s_ps = pool.tile([P, GD], F32,
                                     tag="s_w" if pool is pst else "sh",
                                     name=f"sw{nqc}_{kt}_{h}")
                    nc.tensor.matmul(s_ps[:], kT[:, h, kt * P:(kt + 1) * P],
                                     qT[:, h, nqc * 512:(nqc + 1) * 512],
                                     start=True, stop=True)
                    nc.scalar.activation(ptw[:, h * 512:(h + 1) * 512], s_ps[:],
                                         AF.Exp)
                if ph in (1, 2):
                    hpp = ph - 1
                    ebb = warm_eb[:, kt, :].rearrange(
                        "p (o q) -> p o q", o=1).broadcast_to([P, 2, 512])
                    half = ptw[:, hpp * 1024:(hpp + 1) * 1024].rearrange(
                        "p (i q) -> p i q", i=2)
                    nc.vector.tensor_tensor(half, half, ebb, ALU.mult)

            # process the l-tiles needed last (by the last q-chunk) FIRST so
            # the end-of-loop norm-chain drain only delays tiles nobody needs
            # soon; hold the last 8 tiles' normalize tails back as PE filler
            # for the first attention q-chunk
            q_order = list(range(12, 16)) + list(range(12))
            q_loaded = 2
            held_slots = []
            for i in range(2):
                load_blk('q', q_order[i], d['xqT'])
            for i, lt in enumerate(q_order):
                while q_loaded <= i + 5 and q_loaded < NLT:
                    load_blk('q', q_order[q_loaded], d['xqT'])
                    q_loaded += 1
                blk = blks.pop(('q', lt))
                psum_q = psp.tile([P, GD], F32, tag="psum", name=f"psumq_{lt}")
                for e in range(NET):
                    nc.tensor.matmul(psum_q[:], blk[:, :, e], w_all['wqT'][:, e, :],
                                     start=(e == 0), stop=(e == NET - 1))
                while len(pend_tr) > 2:
                    emit_transposes(pend_tr.pop(0), pst)
                if i in (9, 10) and pend_tr:
                    # drain the remaining deferred transposes early so their
                    # PSUM-freeing copies don't gate the first attention scores
                    emit_transposes(pend_tr.pop(0), pst)
                if i >= 11:
                    for kt_w, ph_w in ((0, i - 11), (1, i - 13)):
                        if 0 <= ph_w <= 2:
                            warm_phase(0, kt_w, ph_w, pst)
                if i >= 8:
                    j = i - 8
                    qk_norm(lt, psum_q[:], d['csq'], qs_t, True,
                            q3_out=q3_hold[:, j, :],
                            nrm_out=nrm_hold[:, j * HPG:(j + 1) * HPG])
                    held_slots.append((lt, j))
                else:
                    q3 = qk_norm(lt, psum_q[:], d['csq'], qs_t, True)
                    pend_tr.append((lt, q3, qT))
            while pend_tr:
                emit_transposes(pend_tr.pop(0), pst)
            proj_ctx.close()

            # attention per q-chunk
            att_ctx = ExitStack()
            asb = att_ctx.enter_context(tc.tile_pool(name="att_sb", bufs=3))
            atp = att_ctx.enter_context(tc.tile_pool(name="att_at", bufs=1))
            aop = att_ctx.enter_context(tc.tile_pool(name="att_o", bufs=4))
            ps_pv = att_ctx.enter_context(tc.tile_pool(name="ps_pv", bufs=1, space="PSUM"))
            ps_sh = att_ctx.enter_context(tc.tile_pool(name="ps_sh", bufs=4, space="PSUM"))

            out_tasks = []
            pend_norm = None
            held_ready = []

            def emit_out_task(task, final=False):
                oqc, attn, lsub, ec = task
                o_ps = ps_sh.tile([P, 512], F32, tag="sh", name=f"o{oqc}_{lsub}_{ec}")
                for h in range(HPG):
                    nc.tensor.matmul(o_ps[:], attn[h][:, lsub * P:(lsub + 1) * P],
                                     wo_sb[:, h, ec * 512:(ec + 1) * 512],
                                     start=(h == 0), stop=(h == HPG - 1))
                o_sb = aop.tile([P, 512], BF16, tag="o_sb", name=f"ob{oqc}_{lsub}_{ec}")
                if final and (lsub + ec) % 2 == 1:
                    nc.scalar.copy(o_sb[:], o_ps[:])
                else:
                    nc.vector.tensor_copy(o_sb[:], o_ps[:])
                nc.sync.dma_start(
                    out[oqc * 512 + lsub * P: oqc * 512 + (lsub + 1) * P,
                        ec * 512:(ec + 1) * 512], o_sb[:])

            def emit_held_dve(slot):
                """Deferred (vector-only) normalize tail for a held Q l-tile."""
                hlt, j = slot
                nv = sq_hold[:, j * HPG:(j + 1) * HPG]
                rcp = asb.tile([P, HPG], F32, tag="hrcp", name=f"hrcp{j}", bufs=2)
                nc.vector.tensor_scalar_max(rcp[:], nv, 1e-12)
                nc.vector.reciprocal(rcp[:], rcp[:])
                nc.vector.tensor_mul(rcp[:], rcp[:], ls_t[:])
                q3 = q3_hold[:, j, :]
                for h in range(HPG):
                    nc.vector.tensor_scalar_mul(q3[:, h * D:(h + 1) * D],
                                                q3[:, h * D:(h + 1) * D],
                                                rcp[:, h:h + 1])
                return (hlt, q3)

            def emit_den_group(den_t, dg, g):
                for h in range(HPG):
                    nc.tensor.matmul(den_t[32 * h:32 * h + 32, :],
                                     ones_b[:, 0:32],
                                     dg[:, h * 512:(h + 1) * 512],
                                     start=(g == 0), stop=(g == NKT // 2 - 1),
                                     tile_position=(0, 32 * h))

            def emit_epilogue(pend):
                """Ln + exp(-x) chain for a q-chunk's 1/den; the scalar table
                loads land in the exp-free warm window of the next chunk."""
                pqc = pend['qc']
                lnd = asb.tile([P, 512], F32, tag="lnd", name=f"lnd{pqc}", bufs=2)
                nc.scalar.activation(lnd[:], pend['den'][:], AF.Ln)
                rcp32s = []
                for i in range(2):
                    r32 = asb.tile([64, 512], F32R, tag=f"rcp32{i}",
                                   name=f"rcp32{pqc}_{i}", bufs=2)
                    nc.scalar.activation(r32[:], lnd[64 * i:64 * i + 64, :],
                                         AF.Exp, scale=-1.0)
                    rcp32s.append(r32)
                pend['rcp32s'] = rcp32s

            def emit_norm(pend):
                """Broadcast 1/den across partitions and normalize pv -> attn;
                returns the outproj tile tasks for this q-chunk."""
                nqc, npvc, rcp32s = pend['qc'], pend['pvc'], pend['rcp32s']
                attn = [atp.tile([P, 512], BF16, tag=f"at{h}", name=f"at{nqc}_{h}",
                                 bufs=2) for h in range(HPG)]
                for h in range(HPG):
                    b_ps = ps_sh.tile([P, 512], F32, tag="sh", name=f"b{nqc}_{h}")
                    off = 32 * (h % 2)
                    nc.tensor.matmul(b_ps[:], ones_r[off:off + 1, :],
                                     rcp32s[h // 2][off:off + 1, :],
                                     start=True, stop=True)
                    rcpb = asb.tile([P, 512], BF16, tag="rcpb", name=f"rb{nqc}_{h}")
                    nc.vector.tensor_copy(rcpb[:], b_ps[:])
                    nc.vector.tensor_mul(attn[h][:], npvc[h], rcpb[:])
                return [(nqc, attn, lsub, ec)
                        for lsub in range(4) for ec in range(4)]

            for qc in range(NQC):
                qsl = slice(qc * 512, (qc + 1) * 512)
                last = qc == NQC - 1
                pv = [ps_pv.tile([P, 512], F32, tag=f"pv{h}", name=f"pv{qc}_{h}")
                      for h in range(HPG)]

                def stage1(kt):
                    eb = asb.tile([P, 512], BF16, tag="eb", name=f"eb{qc}_{kt}")
                    nc.sync.dma_start(eb[:], d['expBT'][kt * P:(kt + 1) * P, qsl])
                    p_t = asb.tile([P, 2048], BF16, tag="p_t", name=f"pt{qc}_{kt}", bufs=5)
                    ebb = eb[:].rearrange("p (o q) -> p o q", o=1).broadcast_to([P, 2, 512])
                    return p_t, ebb

                def score_head(kt, p_t, h):
                    s_ps = ps_sh.tile([P, 512], F32, tag="sh", name=f"sp{qc}_{kt}_{h}")
                    nc.tensor.matmul(s_ps[:], kT[:, h, kt * P:(kt + 1) * P],
                                     qT[:, h, qsl], start=True, stop=True)
                    nc.scalar.activation(p_t[:, h * 512:(h + 1) * 512], s_ps[:], AF.Exp)

                def mult_half(p_t, ebb, hpp):
                    half = p_t[:, hpp * 1024:(hpp + 1) * 1024].rearrange(
                        "p (i q) -> p i q", i=2)
                    nc.vector.tensor_tensor(half, half, ebb, ALU.mult)

                def stage2_pv(kt, p_t):
                    for h in range(HPG):
                        nc.tensor.matmul(pv[h][:], v_sb[:, kt, h * D:(h + 1) * D],
                                         p_t[:, h * 512:(h + 1) * 512],
                                         start=(kt == 0), stop=(kt == NKT - 1))

                # den pre-reduction: pairs of k-tiles are summed on DVE right
                # behind the exp*bias mults; the ones-matmuls run as a burst
                # in the next chunk's warm window (no resident PSUM bank, and
                # the Ln/Exp table thrash lands where the scalar queue is idle)
                dacc = {}
                grp_first = None
                den_t = None

                s2q = []
                for kt in range(NKT):
                    warm = kt < 2
                    if warm:
                        # scores/exp/mult for these tiles ran in the shadow of
                        # the previous phase
                        p_t = warm_pt[:, kt, :]
                    else:
                        p_t, ebb = stage1(kt)
                        score_head(kt, p_t, 0)
                        score_head(kt, p_t, 1)
                        mult_half(p_t, ebb, 0)
                        score_head(kt, p_t, 2)
                    # PE fillers go BEFORE the stage2 pop so the in-order PE
                    # queue has ready work while exp/mult of prev completes
                    if pend_norm is not None and kt == 5:
                        out_tasks.extend(emit_norm(pend_norm))
                        pend_norm = None
                    if kt == 14:
                        # den burst late in the loop: no resident PSUM bank
                        # before this, and the Ln/Exp chain finishes before
                        # the next chunk's scores rotate into these banks
                        den_t = ps_sh.tile([P, 512], F32, tag="sh",
                                           name=f"den{qc}")
                        for g in range(5):
                            emit_den_group(den_t, dacc[g], g)
                    elif kt == 15:
                        emit_den_group(den_t, dacc[5], 5)
                        emit_den_group(den_t, dacc[6], 6)
                        for h in range(HPG):
                            nc.tensor.matmul(
                                den_t[32 * h:32 * h + 32, :],
                                ones_b[:, 0:32],
                                s2q[-1][1][:, h * 512:(h + 1) * 512],
                                start=False, stop=False,
                                tile_position=(0, 32 * h))
                    if held_ready and kt >= 5 and kt % 2 == 1:
                        emit_transposes(held_ready.pop(0) + (qT,), ps_sh)
                    for _ in range(2 if kt < 2 else 1):
                        if out_tasks:
                            emit_out_task(out_tasks.pop(0))
                    # PV runs 4 k-tiles behind scores so the cross-engine
                    # score->exp->mult chain latency plus jitter never stalls
                    # the PE; the deeper drain is free, as it fills the PE
                    # while the epilogue waits on the last mult anyway
                    lag = 4
                    if len(s2q) >= lag:
                        stage2_pv(*s2q.pop(0))
                    if not warm:
                        score_head(kt, p_t, 3)
                        mult_half(p_t, ebb, 1)
                    if held_slots and kt == 2:
                        nc.scalar.activation(sq_hold[:], nrm_hold[:], AF.Sqrt)
                    if held_slots and kt >= 4 and kt % 2 == 0:
                        held_ready.append(emit_held_dve(held_slots.pop(0)))
                    if qc < NQC - 1 and kt >= 12:
                        for kt_w, ph_w in ((0, kt - 12), (1, kt - 13)):
                            if 0 <= ph_w <= 2:
                                warm_phase(qc + 1, kt_w, ph_w, ps_sh)
                    g, r = divmod(kt, 2)
                    if kt >= 14:
                        pass  # raw den matmuls, no pair-add needed
                    elif r == 0:
                        grp_first = p_t
                    else:
                        dg = asb.tile([P, 2048], BF16, tag="dacc",
                                      name=f"dacc{qc}_{g}", bufs=8)
                        dacc[g] = dg
                        nc.vector.tensor_add(dg[:], grp_first[:], p_t[:])
                    s2q.append((kt, p_t))
                for item in s2q:
                    stage2_pv(*item)
                for h in range(HPG):
                    nc.tensor.matmul(den_t[32 * h:32 * h + 32, :],
                                     ones_b[:, 0:32],
                                     s2q[-1][1][:, h * 512:(h + 1) * 512],
                                     start=False, stop=True,
                                     tile_position=(0, 32 * h))
                pend = {'qc': qc, 'den': den_t}
                emit_epilogue(pend)
                if not last:
                    pvc = []
                    for h in range(HPG):
                        c = asb.tile([P, 512], BF16, tag=f"pvc{h}",
                                     name=f"pvc{qc}_{h}", bufs=2)
                        nc.vector.tensor_copy(c[:], pv[h][:])
                        pvc.append(c[:])
                    pend['pvc'] = pvc
                    pend_norm = pend
                else:
                    pend['pvc'] = [pv[h][:] for h in range(HPG)]
                    # drain carried tasks first: they fill the PE while the
                    # Ln/Exp chain and table loads run
                    while out_tasks:
                        emit_out_task(out_tasks.pop(0))
                    out_tasks.extend(emit_norm(pend))
            while out_tasks:
                emit_out_task(out_tasks.pop(0), final=True)
            att_ctx.close()
    nc.compile()
    return nc


# head-dim permutation: within each head, evens first then odds
_PERM = np.empty(GD, np.int64)
for _i in range(GD):
    _h, _j = divmod(_i, D)
    _par, _dd = divmod(_j, D // 2)
    _PERM[_i] = _h * D + 2 * _dd + _par


def _x_relayout(x):
    # [L, E] f32 -> [P, L*NET] bf16 with column index l*NET + e  (4KB runs)
    xt = np.ascontiguousarray(x.T.astype(NP_BF16))        # [E, L]
    xr = xt.reshape(NET, P, L).transpose(1, 2, 0)          # [P, L, NET]
    return np.ascontiguousarray(xr.reshape(P, L * NET))


def _prepare(inputs):
    f32 = np.float32
    inputs_q = np.asarray(inputs["inputs_q"], f32)
    inputs_kv = np.asarray(inputs["inputs_kv"], f32)
    bias = np.asarray(inputs["bias"], f32).reshape(L, L)
    q_sin = np.asarray(inputs["q_sinusoids"], f32)
    k_sin = np.asarray(inputs["k_sinusoids"], f32)
    Wq = np.asarray(inputs["Wq"], f32)
    Wk = np.asarray(inputs["Wk"], f32)
    Wv = np.asarray(inputs["Wv"], f32)
    Wo = np.asarray(inputs["Wo"], f32)
    qns = np.asarray(inputs["q_norm_scale"], f32)
    kns = np.asarray(inputs["k_norm_scale"], f32)
    ls = np.asarray(inputs["logit_scale"], f32)

    apply_qs = not np.all(qns == 1.0)
    apply_ks = not np.all(kns == 1.0)

    bm = bias.max(axis=1, keepdims=True)
    expBT = np.ascontiguousarray(np.exp((bias - bm).T).astype(NP_BF16))
    ls_e = np.exp(np.minimum(ls, LOGIT_SCALE_MAX)).astype(f32)

    per_b = []
    for b in range(B):
        per_b.append(dict(
            xqT=_x_relayout(inputs_q[b]),
            xkvT=_x_relayout(inputs_kv[b]),
            csq=np.ascontiguousarray(np.concatenate(
                [np.tile(q_sin[b][:, 0::2], (1, HPG)),
                 np.tile(q_sin[b][:, 1::2], (1, HPG))], axis=1)),
            csk=np.ascontiguousarray(np.concatenate(
                [np.tile(k_sin[b][:, 0::2], (1, HPG)),
                 np.tile(k_sin[b][:, 1::2], (1, HPG))], axis=1)),
        ))
    per_g = []
    for g in range(G):
        rows = slice(g * GD, (g + 1) * GD)
        per_g.append(dict(
            wqT=np.ascontiguousarray(Wq[rows, :][_PERM, :].T.astype(NP_BF16)),
            wkT=np.ascontiguousarray(Wk[rows, :][_PERM, :].T.astype(NP_BF16)),
            wvT=np.ascontiguousarray(Wv[rows, :].T.astype(NP_BF16)),
            woS=np.ascontiguousarray(Wo[:, rows].T.astype(NP_BF16)),
            ls=np.broadcast_to(ls_e[g * HPG:(g + 1) * HPG][None, :], (P, HPG)).copy(),
        ))

    qs_bc = (np.broadcast_to(np.tile(qns, HPG)[_PERM][None, :], (P, GD)).copy()
             if apply_qs else None)
    ks_bc = (np.broadcast_to(np.tile(kns, HPG)[_PERM][None, :], (P, GD)).copy()
             if apply_ks else None)

    in_maps = []
    for c in range(8):
        b, g = divmod(c, G)
        m = dict(expBT=expBT)
        m.update(per_b[b])
        m.update(per_g[g])
        if apply_qs:
            m['qscale'] = qs_bc
        if apply_ks:
            m['kscale'] = ks_bc
        in_maps.append(m)
    return in_maps, apply_qs, apply_ks


_CACHE = {}


def _get_nc(apply_qs, apply_ks):
    key = (apply_qs, apply_ks)
    if key not in _CACHE:
        _CACHE[key] = _build(apply_qs, apply_ks)
    return _CACHE[key]


def kernel(**inputs) -> np.ndarray:
    in_maps, apply_qs, apply_ks = _prepare(inputs)
    nc = _get_nc(apply_qs, apply_ks)
    res = run_bass_kernel_spmd(nc, in_maps, core_ids=list(range(8)))
    out = np.zeros((B, L, E), np.float32)
    for c in range(8):
        b = c // G
        out[b] += res.results[c]["out"].astype(np.float32)
    return out
